# revision 28
# baseline (speedup 1.0000x reference)
"""Hypergraph conv kernel, v2.

Pipeline (node-sharded, 8 cores):
  phase1: nfw_table[n] = bf16([exp(a_n)*nf_n (128) | exp(a_n) (4) | pad]),
          expl_table[n] = f32 exp(a_n) (for pass B).
  passA:  per edge-run batch: gather nfw rows, ONE reduce -> [U|D] partial,
          cast bf16, scatter-add into U_table[ET,256] (cols 0:132).
  AR:     one bf16 AllReduce of U_table.
  EA:     EAp[e] = bf16((U/D + ef)/D)   (ef projected during passA window)
  passB:  gather EAp rows per incidence, ONE reduce per run group,
          multiply by expl per chunk, scatter-add into y.

All SWDGE gathers/scatters use prepare_only + trigger_dma so descriptor
generation on the Pool engine decouples from the DMA transfers (the v1
bottleneck: GpSimd 75% busy holding each gather ~6.4us while DMA sat 40%).
DMA-completion sems round-robin over 8 handles, mirroring Tile's DMASW
lane rotation (preps are the only Pool-engine DMA instructions emitted).
"""
import numpy as np
from dataclasses import dataclass

import concourse.bass as bass
import concourse.mybir as mybir
import concourse.bacc as bacc
import concourse.tile as tile
import bass_rust
from concourse.library_config import mlp as mlp_library
from concourse._compat import get_trn_type, cdiv

F32 = mybir.dt.float32
BF16 = mybir.dt.bfloat16
I16 = mybir.dt.int16
AX = mybir.AxisListType
ALU = mybir.AluOpType
ACTF = mybir.ActivationFunctionType

ROW = 256          # nfw/U table row width (bf16 elems); cols 0:128 nfw, 128:132 expl
UCOL = 132         # useful cols in nfw/U rows


@dataclass
class Cfg:
    N: int = 100000
    E: int = 25000
    D: int = 128
    H: int = 4
    C: int = 32
    NC: int = 8
    TILE_COLS: int = 30
    NSTAGE: int = 14

    @property
    def NSH(self):
        return self.N // self.NC

    @property
    def NT_ROWS(self):
        return cdiv(self.NSH + 1, 128) * 128

    @property
    def ET_ROWS(self):
        return cdiv(self.E + 1, 128) * 128

    @property
    def DUMMY_NODE(self):
        return self.NSH

    @property
    def JUNK_EDGE(self):
        return self.E


def _runs(keys):
    if len(keys) == 0:
        return (np.zeros(0, np.int64),) * 3
    change = np.flatnonzero(np.diff(keys)) + 1
    starts = np.concatenate([[0], change]).astype(np.int64)
    ends = np.concatenate([change, [len(keys)]]).astype(np.int64)
    return starts, ends - starts, keys[starts].astype(np.int64)


@dataclass
class Sched:
    batches: list          # [(k, tile, c0)]
    groups: list           # [(tile, c0, k, B, b0, chunk)]
    ntiles: int
    nchunks: int
    nbatch: int
    chunk_sizes: list


def _mk_schedule(lens_list, cfg: Cfg) -> Sched:
    sorted_lens = [np.sort(np.asarray(l))[::-1] for l in lens_list]
    nbatch_total = max(cdiv(len(l), 128) for l in sorted_lens)
    batches = []
    for b in range(nbatch_total):
        w = 1
        for ls in sorted_lens:
            if b * 128 < len(ls):
                w = max(w, int(ls[b * 128]))
        batches.append(w)
    assert max(batches) <= cfg.TILE_COLS, \
        f"run length {max(batches)} > TILE_COLS"
    placed = []
    t, c = 0, 0
    for k in batches:
        if c + k > cfg.TILE_COLS:
            t += 1
            c = 0
        placed.append((k, t, c))
        c += k
    ntiles = t + 1 if placed else 1
    nbatch = len(placed)
    nchunks = cdiv(nbatch, cfg.NSTAGE)
    chunk_sizes = [min(cfg.NSTAGE, nbatch - i * cfg.NSTAGE) for i in range(nchunks)]
    groups = []
    for bi, (k, t, c0) in enumerate(placed):
        ch = bi // cfg.NSTAGE
        if groups and groups[-1][0] == t and groups[-1][2] == k \
                and groups[-1][5] == ch \
                and groups[-1][1] + groups[-1][2] * groups[-1][3] == c0 \
                and groups[-1][4] + groups[-1][3] == bi:
            t0, c0g, kg, B, b0, chg = groups[-1]
            groups[-1] = (t0, c0g, kg, B + 1, b0, chg)
        else:
            groups.append((t, c0, k, 1, bi, ch))
    return Sched(placed, groups, ntiles, nchunks, nbatch, chunk_sizes)


def _wrap16(flat):
    assert len(flat) % 16 == 0
    b = flat.reshape(-1, 16).T.astype(np.int16)
    return np.tile(b, (8, 1))


SUBMAX = 15


def _subcols(n):
    return [(i, min(SUBMAX, n - i)) for i in range(0, n, SUBMAX)]


def _mk_streams(sched: Sched, starts, lens, gvals, svals, runvals,
                dummy_g, junk_s, dummy_run, cfg: Cfg):
    TC = cfg.TILE_COLS
    g_arr = np.full((sched.ntiles, TC, 128), dummy_g, np.int64)
    s_arr = np.full((sched.nbatch, 128), junk_s, np.int64)
    r_arr = np.full((sched.nbatch, 128), dummy_run, np.int64)
    order = np.argsort(-lens, kind="stable") if len(lens) else np.zeros(0, np.int64)
    for bi, (k, t, c0) in enumerate(sched.batches):
        idxs = order[bi * 128:(bi + 1) * 128]
        nr = len(idxs)
        if nr:
            st = starts[idxs]
            kr = lens[idxs]
            assert kr[0] <= k
            for kk in np.unique(kr):
                sel = np.flatnonzero(kr == kk)
                gm = gvals[st[sel][None, :] + np.arange(kk)[:, None]]
                g_arr[t, c0:c0 + kk, sel] = gm.T
            s_arr[bi, :nr] = svals[idxs]
            r_arr[bi, :nr] = runvals[idxs]
    g_idx = np.concatenate(
        [_wrap16(g_arr[t, c0:c0 + cc].reshape(-1))
         for t in range(sched.ntiles) for (c0, cc) in _subcols(TC)], axis=1)
    sc_blocks = []
    off = 0
    for nb in sched.chunk_sizes:
        sc_blocks.append(_wrap16(s_arr[off:off + nb].reshape(-1)))
        off += nb
    sc_idx = np.concatenate(sc_blocks, axis=1)
    r_idx = np.concatenate(
        [_wrap16(r_arr[b0:b0 + bb].reshape(-1))
         for (b0, bb) in _subcols(sched.nbatch)], axis=1)
    return g_idx, sc_idx, r_idx


def build_plan(node_idx, edge_idx, cfg: Cfg):
    node_idx = np.asarray(node_idx).astype(np.int64)
    edge_idx = np.asarray(edge_idx).astype(np.int64)
    percore = []
    for m in range(cfg.NC):
        sel = np.flatnonzero(node_idx // cfg.NSH == m)
        nl = node_idx[sel] - m * cfg.NSH
        eg = edge_idx[sel]
        sA, lA, vA = _runs(eg)
        oB = np.argsort(nl, kind="stable")
        nB = nl[oB]
        eB = eg[oB]
        sB, lB, vB = _runs(nB)
        percore.append(dict(nl=nl, eg=eg, sA=sA, lA=lA, vA=vA,
                            eB=eB, sB=sB, lB=lB, vB=vB))
    schedA = _mk_schedule([c["lA"] for c in percore], cfg)
    schedB = _mk_schedule([c["lB"] for c in percore], cfg)
    streams = []
    for c in percore:
        gA, scA, _ = _mk_streams(
            schedA, c["sA"], c["lA"],
            gvals=c["nl"], svals=c["vA"], runvals=c["vA"],
            dummy_g=cfg.DUMMY_NODE, junk_s=cfg.JUNK_EDGE,
            dummy_run=cfg.JUNK_EDGE, cfg=cfg)
        gB, scB, rB = _mk_streams(
            schedB, c["sB"], c["lB"],
            gvals=c["eB"], svals=c["vB"], runvals=c["vB"],
            dummy_g=cfg.JUNK_EDGE, junk_s=cfg.DUMMY_NODE,
            dummy_run=cfg.DUMMY_NODE, cfg=cfg)
        streams.append(dict(gA=gA, scA=scA, gB=gB, scB=scB, rB=rB))
    return schedA, schedB, streams


def _ap(t_ap, off, dims):
    base = t_ap
    part = base.ap[0]
    return bass_rust.AP(base.tensor, base.offset + off, [part] + dims)


def build_bass(cfg: Cfg, schedA: Sched, schedB: Sched, replica_groups):
    import os
    _stops = ["init", "phase1", "passA", "coll", "ea", "full"]
    _stop = _stops.index(os.environ.get("GNN_STOP", "full"))
    TC, NS = cfg.TILE_COLS, cfg.NSTAGE
    H, C = cfg.H, cfg.C
    D = cfg.D
    NT, ET = cfg.NT_ROWS, cfg.ET_ROWS
    NSH, E = cfg.NSH, cfg.E
    n_a_node = NT // 128
    n_a_edge = ET // 128

    nc = bacc.Bacc(get_trn_type() or "TRN2", target_bir_lowering=False, debug=False,
                   num_swdge_queues=4)

    # ---- I/O ----
    xT = nc.dram_tensor("xT", [D, NT], F32, kind="ExternalInput")
    haT = nc.dram_tensor("haT", [D, ET], F32, kind="ExternalInput")
    Wn = nc.dram_tensor("Wn", [D, H * C], F32, kind="ExternalInput")
    We = nc.dram_tensor("We", [D, H * C], F32, kind="ExternalInput")
    attn = nc.dram_tensor("attn", [128, H * C], F32, kind="ExternalInput")
    bias_t = nc.dram_tensor("bias_t", [128, 16 * H * C], F32, kind="ExternalInput")
    gA_i = nc.dram_tensor("gA_i", [128, schedA.ntiles * TC * 8], I16, kind="ExternalInput")
    scA_i = nc.dram_tensor("scA_i", [128, schedA.nbatch * 8], I16, kind="ExternalInput")
    gB_i = nc.dram_tensor("gB_i", [128, schedB.ntiles * TC * 8], I16, kind="ExternalInput")
    scB_i = nc.dram_tensor("scB_i", [128, schedB.nbatch * 8], I16, kind="ExternalInput")
    rB_i = nc.dram_tensor("rB_i", [128, schedB.nbatch * 8], I16, kind="ExternalInput")
    y = nc.dram_tensor("y", [NT, H * C], F32, kind="ExternalOutput")
    _dbg = os.environ.get("GNN_DEBUG_OUTS", "0") == "1"
    if _dbg:
        nfw_dbg = nc.dram_tensor("nfw_dbg", [NT, ROW], BF16, kind="ExternalOutput")
        U_dbg = nc.dram_tensor("U_dbg", [ET, ROW], BF16, kind="ExternalOutput")
        Ured_dbg = nc.dram_tensor("Ured_dbg", [ET, ROW], BF16, kind="ExternalOutput")
        EAp_dbg = nc.dram_tensor("EAp_dbg", [ET, 128], BF16, kind="ExternalOutput")

    # ---- internal DRAM ----
    nfw_table = nc.dram_tensor("nfw_table", [NT, ROW], BF16)
    expl_table = nc.dram_tensor("expl_table", [NT, 64], F32)
    ef_table = nc.dram_tensor("ef_table", [ET, 128], BF16)
    U_table = nc.dram_tensor("U_table", [ET, ROW], BF16)
    U_red = nc.dram_tensor("U_red", [ET, ROW], BF16, addr_space="Shared")
    EAp = nc.dram_tensor("EAp", [ET, 128], BF16)

    nfw_v = nfw_table[:].rearrange("(a p) c -> p a c", p=128)
    expl_v = expl_table[:].rearrange("(a p) c -> p a c", p=128)
    ef_v = ef_table[:].rearrange("(a p) c -> p a c", p=128)
    U_v = U_table[:].rearrange("(a p) c -> p a c", p=128)
    Ured_v = U_red[:].rearrange("(a p) c -> p a c", p=128)
    EAp_v = EAp[:].rearrange("(a p) c -> p a c", p=128)
    y_v = y[:].rearrange("(a p) c -> p a c", p=128)

    # Manual DMA-completion tracking for prepare_only SWDGE ops.
    #
    # Tile's DMASW machinery cannot order data-consumers after a prepped
    # DMA's completion: it pre-bumps the lane sem (InstIncSwdgeSem +16)
    # BEFORE the prep, so its consumer waits are satisfied early. We
    # therefore bake our OWN sems into the descriptors (sem=) and attach
    # explicit waits to the first consumer instruction of each prepped
    # transfer via BassInstruction.wait_op. All preps go on queue 0 (the
    # queue Tile's pre-bumps target; one queue still feeds all 16 SDMA
    # engines). Tile's own (early-satisfied) waits remain, harmlessly.
    #
    # Everything ELSE is sound through Tile: a prep defers its table-read /
    # table-write deps to its trigger (tracked against HWDGE/engine sems),
    # and prep-vs-reader buffer WAW gates the prep on tracked engine ticks.
    # Manual additions needed: (1) consumer waits after gather DMAs,
    # (2) WAR on scatter-source staging reuse, (3) scatter-completion
    # gates before the collective / kernel end.

    class _SemCtr:
        def __init__(self, name, n):
            self.sems = [nc.alloc_semaphore(f"{name}{i}") for i in range(n)]
            self.vals = [0] * n
            self.i = 0

        def acquire(self):
            s = self.i % len(self.sems)
            self.i += 1
            return s

        def bump(self, slot):
            self.vals[slot] += 16
            return self.sems[slot]

        def cur(self, slot):
            return self.sems[slot], self.vals[slot]

    with tile.TileContext(nc) as tc:
        dma_sems = tc.sems.swdge_block()
        with tc.tile_pool(name="const", bufs=1) as cpool:
            nc.gpsimd.load_library(mlp_library)
            Wn_sb = cpool.tile([D, H * C], F32)
            We_sb = cpool.tile([D, H * C], F32)
            attn_sb = cpool.tile([128, H * C], F32)
            bias_sb = cpool.tile([128, 16 * H * C], F32)
            zbf = cpool.tile([128, 16 * ROW], BF16)
            nc.sync.dma_start(Wn_sb[:], Wn[:])
            nc.sync.dma_start(We_sb[:], We[:])
            nc.sync.dma_start(attn_sb[:], attn[:])
            nc.sync.dma_start(bias_sb[:], bias_t[:])
            nc.gpsimd.memset(zbf[:], 0.0)

            # ---------- init: U=0 (scalar HWDGE queue), y=bias ----------
            zv = zbf[:].rearrange("p (a c) -> p a c", c=ROW)
            for a0 in range(0, n_a_edge, 16):
                aa = min(16, n_a_edge - a0)
                nc.scalar.dma_start(U_v[:, a0:a0 + aa, :], zv[:, :aa, :])
            bv = bias_sb[:].rearrange("p (a c) -> p a c", c=H * C)
            for a0 in range(0, n_a_node, 16):
                aa = min(16, n_a_node - a0)
                nc.scalar.dma_start(y_v[:, a0:a0 + aa, :], bv[:, :aa, :])

            # ---------- phase 1: node projection -> nfw + expl ----------
            if _stop >= 1:
                with (tc.tile_pool(name="p1", bufs=2) as p1,
                    tc.tile_pool(name="p1s", bufs=2) as p1s,
                    tc.tile_pool(name="ps", bufs=4, space="PSUM") as psp):
                  CHK = 16
                  for a0 in range(0, n_a_node, CHK):
                      aa = min(CHK, n_a_node - a0)
                      xc = p1.tile([D, CHK * 128], F32, tag="xc")
                      nc.sync.dma_start(xc[:, :aa * 128], xT[:, a0 * 128:(a0 + aa) * 128])
                      nfst = p1s.tile([128, CHK, 128], F32, tag="nfst")
                      for i in range(aa):
                          mm = psp.tile([128, 128], F32, tag="mm")
                          nc.tensor.matmul(mm[:], xc[:, i * 128:(i + 1) * 128], Wn_sb[:],
                                           start=True, stop=True)
                          nc.vector.tensor_copy(nfst[:, i, :], mm[:])
                      tmp1 = p1s.tile([128, CHK, 128], F32, tag="tmp1")
                      nc.vector.tensor_tensor(
                          out=tmp1[:, :aa, :].rearrange("p a (h c) -> p a h c", h=H),
                          in0=nfst[:, :aa, :].rearrange("p a (h c) -> p a h c", h=H),
                          in1=attn_sb[:].rearrange("p (h c) -> p h c", h=H)
                              .unsqueeze(1).broadcast_to([128, aa, H, C]),
                          op=ALU.mult)
                      praw = p1s.tile([128, CHK, H], F32, tag="praw")
                      nc.vector.tensor_reduce(
                          out=praw[:, :aa, :],
                          in_=tmp1[:, :aa, :].rearrange("p a (h c) -> p a h c", h=H),
                          axis=AX.X, op=ALU.add)
                      est = p1s.tile([128, CHK, 64], F32, tag="est")
                      nc.gpsimd.memset(est[:], 0.0)
                      nc.scalar.activation(est[:, :aa, 0:H], praw[:, :aa, :], ACTF.Exp)
                      nrow = p1s.tile([128, CHK, ROW], BF16, tag="nrow")
                      nc.gpsimd.memset(nrow[:], 0.0)
                      nc.vector.tensor_tensor(
                          out=nrow[:, :aa, 0:128].rearrange("p a (h c) -> p a h c", h=H),
                          in0=nfst[:, :aa, :].rearrange("p a (h c) -> p a h c", h=H),
                          in1=est[:, :aa, 0:H].unsqueeze(3).broadcast_to([128, aa, H, C]),
                          op=ALU.mult)
                      nc.vector.tensor_copy(nrow[:, :aa, 128:UCOL], est[:, :aa, 0:H])
                      nc.sync.dma_start(nfw_v[:, a0:a0 + aa, :], nrow[:, :aa, :])
                      nc.sync.dma_start(expl_v[:, a0:a0 + aa, :], est[:, :aa, :])
                  # zero dummy/pad rows (incl. expl cols -> no D contribution)
                  nc.sync.dma_start(nfw_table[NSH:NT, :], zbf[0:NT - NSH, 0:ROW])

            # ---------- pass A ----------
            if _stop >= 2:
                gsemA = _SemCtr("gA_dma", 8)
                scsemA = _SemCtr("scA_dma", 1)
                # staging bufs=4 makes buffer-reuse WAR sound with NO manual
                # waits: a chunk-ch writer is RAW-gated on reduces, which wait
                # gathers enqueued (queue-0 FIFO) after flush(ch-s-1), s<=3
                # being the max chunks one tile spans; FIFO order then implies
                # scatter(ch-4) has fully drained.
                _span = {}
                for g in schedA.groups:
                    _span.setdefault(g[0], []).append(g[5])
                assert all(max(v) - min(v) <= 2 for v in _span.values()), "tile spans >3 chunks"
                with (tc.tile_pool(name="gA", bufs=2) as gpool,
                    tc.tile_pool(name="stA", bufs=4) as spool,
                    tc.tile_pool(name="idxA", bufs=1) as ipool):
                  gA_sb = ipool.tile([128, schedA.ntiles * TC * 8], I16)
                  scA_sb = ipool.tile([128, schedA.nbatch * 8], I16)
                  nc.sync.dma_start(gA_sb[:], gA_i[:])
                  nc.sync.dma_start(scA_sb[:], scA_i[:])

                  groups_by_tile = {}
                  for g in schedA.groups:
                      groups_by_tile.setdefault(g[0], []).append(g)

                  cur_chunk = [0]
                  stag = {}
                  sc_off = [0]

                  def open_chunk():
                      stag["U"] = spool.tile([128, NS, UCOL], F32, tag="ustag", name="ustag")

                  def flush_chunk():
                      ch = cur_chunk[0]
                      nb = schedA.chunk_sizes[ch]
                      # full 512B rows (scatter payloads must stay on the
                      # proven 256B-multiple geometry); pad cols add zero.
                      # Pads are zeroed once per buffer instance (first 4
                      # chunks) and never dirtied afterwards.
                      ubf = spool.tile([128, NS, ROW], BF16, tag="ubf", name="ubf")
                      if ch < 4:
                          nc.gpsimd.memset(ubf[:, :, UCOL:], 0.0)
                      nc.vector.tensor_copy(ubf[:, :nb, 0:UCOL], stag["U"][:, :nb, :])
                      sem = scsemA.bump(0)
                      nc.gpsimd.dma_scatter_add(
                          U_table[:], ubf[:, :nb, :],
                          scA_sb[:, sc_off[0]:sc_off[0] + nb * 8],
                          nb * 128, nb * 128, ROW,
                          prepare_only=True, sem=sem,
                          single_packet=False, queue_num=0)
                      nc.gpsimd.trigger_dma(count=None, queue_num=0)
                      sc_off[0] += nb * 8
                      cur_chunk[0] += 1

                  open_chunk()
                  for t in range(schedA.ntiles):
                      G = gpool.tile([128, TC, ROW], BF16, tag="G")
                      slot = gsemA.acquire()
                      for (c0s, cc) in _subcols(TC):
                          sem = gsemA.bump(slot)
                          nc.gpsimd.dma_gather(
                              G[:, c0s:c0s + cc, :], nfw_table[:],
                              gA_sb[:, t * TC * 8 + c0s * 8:
                                    t * TC * 8 + (c0s + cc) * 8],
                              cc * 128, cc * 128, ROW,
                              prepare_only=True, sem=sem,
                              single_packet=False, queue_num=0)
                          nc.gpsimd.trigger_dma(count=None, queue_num=0)
                      for (_, c0, k, B, b0, ch) in groups_by_tile.get(t, []):
                          if ch != cur_chunk[0]:
                              flush_chunk()
                              open_chunk()
                          bpos = b0 - ch * NS
                          red = nc.vector.tensor_reduce(
                              out=stag["U"][:, bpos:bpos + B, :],
                              in_=_ap(G[:], c0 * ROW, [[k * ROW, B], [1, UCOL], [ROW, k]]),
                              axis=AX.X, op=ALU.add)
                          # scheduler may reorder same-engine reduces, so
                          # every consumer carries the gather-complete wait
                          s, v = gsemA.cur(slot)
                          red.wait_op(s, v, "sem-ge")
                  flush_chunk()

            # ---------- edge projection (overlaps passA tail / AR) ----------
            if _stop >= 1:
                with (tc.tile_pool(name="pe", bufs=2) as pe,
                    tc.tile_pool(name="pes", bufs=2) as pes,
                    tc.tile_pool(name="ps2", bufs=4, space="PSUM") as psp2):
                  CHK = 16
                  for a0 in range(0, n_a_edge, CHK):
                      aa = min(CHK, n_a_edge - a0)
                      hc = pe.tile([D, CHK * 128], F32, tag="hc")
                      nc.sync.dma_start(hc[:, :aa * 128], haT[:, a0 * 128:(a0 + aa) * 128])
                      efst = pes.tile([128, CHK, 128], BF16, tag="efst")
                      for i in range(aa):
                          mm = psp2.tile([128, 128], F32, tag="mm")
                          nc.tensor.matmul(mm[:], hc[:, i * 128:(i + 1) * 128], We_sb[:],
                                           start=True, stop=True)
                          nc.vector.tensor_copy(efst[:, i, :], mm[:])
                      nc.sync.dma_start(ef_v[:, a0:a0 + aa, :], efst[:, :aa, :])

            # ---------- pass B prologue: index/expl loads (overlap AR) ----------
            if _stop >= 5:
                pb_stack = [tc.tile_pool(name="gB", bufs=2),
                            tc.tile_pool(name="stB", bufs=4),
                            tc.tile_pool(name="idxB", bufs=1)]
                gpoolB, spoolB, ipoolB = [p.__enter__() for p in pb_stack]
                gB_sb = ipoolB.tile([128, schedB.ntiles * TC * 8], I16)
                scB_sb = ipoolB.tile([128, schedB.nbatch * 8], I16)
                rB_sb = ipoolB.tile([128, schedB.nbatch * 8], I16)
                nc.sync.dma_start(gB_sb[:], gB_i[:])
                nc.sync.dma_start(scB_sb[:], scB_i[:])
                nc.sync.dma_start(rB_sb[:], rB_i[:])
                exsem = _SemCtr("explg_dma", 1)
                explg = ipoolB.tile([128, schedB.nbatch, 64], F32)
                for (b0s, bb) in _subcols(schedB.nbatch):
                    sem = exsem.bump(0)
                    nc.gpsimd.dma_gather(
                        explg[:, b0s:b0s + bb, :], expl_table[:],
                        rB_sb[:, b0s * 8:(b0s + bb) * 8],
                        bb * 128, bb * 128, 64,
                        prepare_only=True, sem=sem,
                        single_packet=False, queue_num=0)
                    nc.gpsimd.trigger_dma(count=None, queue_num=0)

            # ---------- collective ----------
            if _stop >= 3:
                coll = nc.gpsimd.collective_compute(
                    "AllReduce", ALU.add, replica_groups=replica_groups,
                    ins=[U_table[:]], outs=[U_red[:]])
                if _stop >= 2:
                    # gate on all pass-A U scatter DMAs having drained
                    s, v = scsemA.cur(0)
                    coll.wait_op(s, v, "sem-ge")

            # ---------- EA' = (U/D + ef)/D ----------
            if _stop >= 4:
                with tc.tile_pool(name="ea", bufs=2) as eap:
                  for a0 in range(0, n_a_edge, 16):
                      aa = min(16, n_a_edge - a0)
                      uc = eap.tile([128, 16, ROW], BF16, tag="uc")
                      efc = eap.tile([128, 16, 128], BF16, tag="efc")
                      nc.sync.dma_start(uc[:, :aa, :], Ured_v[:, a0:a0 + aa, :])
                      nc.sync.dma_start(efc[:, :aa, :], ef_v[:, a0:a0 + aa, :])
                      dv = eap.tile([128, 16, H], F32, tag="dv")
                      nc.vector.tensor_copy(dv[:, :aa, :], uc[:, :aa, 128:UCOL])
                      nc.vector.tensor_scalar_add(dv[:, :aa, :], dv[:, :aa, :], 1e-30)
                      inv = eap.tile([128, 16, H], F32, tag="inv")
                      nc.vector.reciprocal(inv[:, :aa, :], dv[:, :aa, :])
                      inv_b = inv[:, :aa, :].unsqueeze(3).broadcast_to([128, aa, H, C])
                      t1 = eap.tile([128, 16, 128], F32, tag="t1")
                      nc.vector.tensor_tensor(
                          out=t1[:, :aa, :].rearrange("p a (h c) -> p a h c", h=H),
                          in0=uc[:, :aa, 0:128].rearrange("p a (h c) -> p a h c", h=H),
                          in1=inv_b, op=ALU.mult)
                      nc.vector.tensor_tensor(out=t1[:, :aa, :], in0=t1[:, :aa, :],
                                              in1=efc[:, :aa, :], op=ALU.add)
                      eab = eap.tile([128, 16, 128], BF16, tag="eab")
                      nc.vector.tensor_tensor(
                          out=eab[:, :aa, :].rearrange("p a (h c) -> p a h c", h=H),
                          in0=t1[:, :aa, :].rearrange("p a (h c) -> p a h c", h=H),
                          in1=inv_b, op=ALU.mult)
                      nc.sync.dma_start(EAp_v[:, a0:a0 + aa, :], eab[:, :aa, :])
                  nj = ET - E
                  nc.sync.dma_start(EAp[E:ET, :], zbf[0:nj, 0:128])

            if _dbg:
                if _stop >= 1:
                    nc.sync.dma_start(nfw_dbg[:], nfw_table[:])
                if _stop >= 2:
                    d2 = nc.sync.dma_start(U_dbg[:], U_table[:])
                    s, v = scsemA.cur(0)
                    d2.wait_op(s, v, "sem-ge")
                if _stop >= 3:
                    nc.sync.dma_start(Ured_dbg[:], U_red[:])
                if _stop >= 4:
                    nc.sync.dma_start(EAp_dbg[:], EAp[:])

            # ---------- pass B ----------
            if _stop >= 5:
                gsemB = _SemCtr("gB_dma", 8)
                ysem = _SemCtr("scB_dma", 1)
                groups_by_tileB = {}
                for g in schedB.groups:
                    groups_by_tileB.setdefault(g[0], []).append(g)
                cur_chunkB = [0]
                stagB = {}
                sc_offB = [0]
                first_flushB = [True]

                def open_chunkB():
                    stagB["Y"] = spoolB.tile([128, NS, 128], F32, tag="ystag", name="ystag")

                def flush_chunkB():
                    ch = cur_chunkB[0]
                    nb = schedB.chunk_sizes[ch]
                    yst = stagB["Y"]
                    mul = nc.vector.tensor_tensor(
                        out=_ap(yst[:], 0, [[128, nb], [32, H], [1, C]]),
                        in0=_ap(yst[:], 0, [[128, nb], [32, H], [1, C]]),
                        in1=_ap(explg[:], ch * NS * 64, [[64, nb], [1, H], [0, C]]),
                        op=ALU.mult)
                    if first_flushB[0]:
                        s, v = exsem.cur(0)
                        mul.wait_op(s, v, "sem-ge")
                        first_flushB[0] = False
                    sem = ysem.bump(0)
                    nc.gpsimd.dma_scatter_add(
                        y[:], yst[:, :nb, :],
                        scB_sb[:, sc_offB[0]:sc_offB[0] + nb * 8],
                        nb * 128, nb * 128, 128,
                        prepare_only=True, sem=sem,
                        single_packet=False, queue_num=0)
                    nc.gpsimd.trigger_dma(count=None, queue_num=0)
                    sc_offB[0] += nb * 8
                    cur_chunkB[0] += 1

                open_chunkB()
                for t in range(schedB.ntiles):
                    G = gpoolB.tile([128, TC, 128], BF16, tag="G")
                    slot = gsemB.acquire()
                    for (c0s, cc) in _subcols(TC):
                        sem = gsemB.bump(slot)
                        nc.gpsimd.dma_gather(
                            G[:, c0s:c0s + cc, :], EAp[:],
                            gB_sb[:, t * TC * 8 + c0s * 8:
                                  t * TC * 8 + (c0s + cc) * 8],
                            cc * 128, cc * 128, 128,
                            prepare_only=True, sem=sem,
                            single_packet=False, queue_num=0)
                        nc.gpsimd.trigger_dma(count=None, queue_num=0)
                    for (_, c0, k, B, b0, ch) in groups_by_tileB.get(t, []):
                        if ch != cur_chunkB[0]:
                            flush_chunkB()
                            open_chunkB()
                        bpos = b0 - ch * NS
                        red = nc.vector.tensor_reduce(
                            out=stagB["Y"][:, bpos:bpos + B, :],
                            in_=_ap(G[:], c0 * 128, [[k * 128, B], [1, 128], [128, k]]),
                            axis=AX.X, op=ALU.add)
                        s, v = gsemB.cur(slot)
                        red.wait_op(s, v, "sem-ge")
                flush_chunkB()
                # terminal guard: kernel end must not race the last y
                # scatter DMAs; hang the wait on a real consumer of yst.
                guard = spoolB.tile([128, 1, 128], F32, tag="guard", name="guard")
                g = nc.vector.tensor_copy(guard[:, 0, :], stagB["Y"][:, 0, :])
                s, v = ysem.cur(0)
                g.wait_op(s, v, "sem-ge")
                for p in reversed(pb_stack):
                    p.__exit__(None, None, None)
    nc.compile()
    return nc


def host_inputs(cfg: Cfg, x, ha, W_node, W_edge, attn_l, bias, streams):
    x = np.asarray(x, np.float32)
    ha = np.asarray(ha, np.float32)
    W_node = np.asarray(W_node, np.float32)
    W_edge = np.asarray(W_edge, np.float32)
    attn_flat = np.asarray(attn_l, np.float32).reshape(-1)
    bias = np.asarray(bias, np.float32).reshape(-1)
    attn_rep = np.tile(attn_flat[None, :], (128, 1))
    bias_t = np.tile(bias[None, :], (128, 16))
    ha_pad = np.zeros((cfg.ET_ROWS, cfg.D), np.float32)
    ha_pad[:cfg.E] = ha
    haT = np.ascontiguousarray(ha_pad.T)
    in_maps = []
    for m in range(cfg.NC):
        xs = np.zeros((cfg.NT_ROWS, cfg.D), np.float32)
        xs[:cfg.NSH] = x[m * cfg.NSH:(m + 1) * cfg.NSH]
        st = streams[m]
        in_maps.append({
            "xT": np.ascontiguousarray(xs.T),
            "haT": haT,
            "Wn": W_node, "We": W_edge,
            "attn": attn_rep, "bias_t": bias_t,
            "gA_i": st["gA"], "scA_i": st["scA"],
            "gB_i": st["gB"], "scB_i": st["scB"], "rB_i": st["rB"],
        })
    return in_maps


# ======================== public entry point ========================
_CFG = Cfg()
LAST_RESULTS = None


def _install_axon_ntff_shim():
    import sys, types, ctypes, contextlib
    import concourse.bass_utils as bu
    bu.upload_artifacts = lambda d: str(d)
    try:
        from antenv.axon_hooks import get_axon_ntff_profile_hook  # noqa
        return
    except ImportError:
        pass
    so_path = "/opt/axon/libaxon_pjrt.so"
    try:
        lib = ctypes.CDLL(so_path)
    except OSError:
        return
    if not hasattr(lib, "axon_start_nrt_profile"):
        return
    lib.axon_start_nrt_profile.argtypes = [ctypes.POINTER(ctypes.c_int64),
                                           ctypes.c_size_t]
    lib.axon_start_nrt_profile.restype = ctypes.c_int64
    lib.axon_stop_nrt_profile.argtypes = [ctypes.c_char_p]
    lib.axon_stop_nrt_profile.restype = ctypes.c_int64

    @contextlib.contextmanager
    def _hook(output_dir, device_ids):
        import jax
        jax.devices()
        if device_ids:
            ids = (ctypes.c_int64 * len(device_ids))(*device_ids)
            rc = lib.axon_start_nrt_profile(ids, len(device_ids))
        else:
            rc = lib.axon_start_nrt_profile(None, 0)
        if rc != 0:
            raise RuntimeError(f"axon_start_nrt_profile rc={rc}")
        try:
            yield
        finally:
            n = lib.axon_stop_nrt_profile(str(output_dir).encode())
            print(f"ntff profile: {n} file(s) -> {output_dir}")

    mod = types.ModuleType("antenv.axon_hooks")
    mod.get_axon_ntff_profile_hook = lambda: _hook
    mod.set_axon_ntff_profile_hook = lambda h: None
    sys.modules["antenv.axon_hooks"] = mod


def kernel(**inputs) -> np.ndarray:
    import os
    from concourse.bass_utils import run_bass_kernel_spmd
    cfg = _CFG
    x = np.asarray(inputs["x"], np.float32)
    ha = np.asarray(inputs["hyperedge_attr"], np.float32)
    node_idx = np.asarray(inputs["node_idx"]).astype(np.int64)
    edge_idx = np.asarray(inputs["edge_idx"]).astype(np.int64)
    schedA, schedB, streams = build_plan(node_idx, edge_idx, cfg)
    nc = build_bass(cfg, schedA, schedB, [list(range(cfg.NC))])
    in_maps = host_inputs(cfg, x, ha, inputs["W_node"], inputs["W_edge"],
                          inputs["attn_l"], inputs["bias"], streams)
    trace = os.environ.get("GNN_TRACE", "0") == "1"
    if trace:
        _install_axon_ntff_shim()
    res = run_bass_kernel_spmd(nc, in_maps, list(range(cfg.NC)), trace=trace)
    global LAST_RESULTS
    LAST_RESULTS = res
    out = np.concatenate(
        [np.asarray(res.results[m]["y"])[:cfg.NSH] for m in range(cfg.NC)], axis=0)
    return np.ascontiguousarray(out, dtype=np.float32)


# revision 29
# speedup vs baseline: 1.1128x; 1.1128x over previous
"""Hypergraph conv kernel, v2.

Pipeline (node-sharded, 8 cores):
  phase1: nfw_table[n] = bf16([exp(a_n)*nf_n (128) | exp(a_n) (4) | pad]),
          expl_table[n] = f32 exp(a_n) (for pass B).
  passA:  per edge-run batch: gather nfw rows, ONE reduce -> [U|D] partial,
          cast bf16, scatter-add into U_table[ET,256] (cols 0:132).
  AR:     one bf16 AllReduce of U_table.
  EA:     EAp[e] = bf16((U/D + ef)/D)   (ef projected during passA window)
  passB:  gather EAp rows per incidence, ONE reduce per run group,
          multiply by expl per chunk, scatter-add into y.

All SWDGE gathers/scatters use prepare_only + trigger_dma so descriptor
generation on the Pool engine decouples from the DMA transfers (the v1
bottleneck: GpSimd 75% busy holding each gather ~6.4us while DMA sat 40%).
DMA-completion sems round-robin over 8 handles, mirroring Tile's DMASW
lane rotation (preps are the only Pool-engine DMA instructions emitted).
"""
import numpy as np
from dataclasses import dataclass

import concourse.bass as bass
import concourse.mybir as mybir
import concourse.bacc as bacc
import concourse.tile as tile
import bass_rust
from concourse.library_config import mlp as mlp_library
from concourse._compat import get_trn_type, cdiv

F32 = mybir.dt.float32
BF16 = mybir.dt.bfloat16
I16 = mybir.dt.int16
AX = mybir.AxisListType
ALU = mybir.AluOpType
ACTF = mybir.ActivationFunctionType

ROW = 256          # nfw/U table row width (bf16 elems); cols 0:128 nfw, 128:132 expl
UCOL = 132         # useful cols in nfw/U rows


@dataclass
class Cfg:
    N: int = 100000
    E: int = 25000
    D: int = 128
    H: int = 4
    C: int = 32
    NC: int = 8
    TILE_COLS: int = 30
    NSTAGE: int = 14

    @property
    def NSH(self):
        return self.N // self.NC

    @property
    def NT_ROWS(self):
        return cdiv(self.NSH + 1, 128) * 128

    @property
    def ET_ROWS(self):
        return cdiv(self.E + 1, 128) * 128

    @property
    def DUMMY_NODE(self):
        return self.NSH

    @property
    def JUNK_EDGE(self):
        return self.E


def _runs(keys):
    if len(keys) == 0:
        return (np.zeros(0, np.int64),) * 3
    change = np.flatnonzero(np.diff(keys)) + 1
    starts = np.concatenate([[0], change]).astype(np.int64)
    ends = np.concatenate([change, [len(keys)]]).astype(np.int64)
    return starts, ends - starts, keys[starts].astype(np.int64)


@dataclass
class Sched:
    batches: list          # [(k, tile, c0)]
    groups: list           # [(tile, c0, k, B, b0, chunk)]
    ntiles: int
    nchunks: int
    nbatch: int
    chunk_sizes: list


def _mk_schedule(lens_list, cfg: Cfg) -> Sched:
    sorted_lens = [np.sort(np.asarray(l))[::-1] for l in lens_list]
    nbatch_total = max(cdiv(len(l), 128) for l in sorted_lens)
    batches = []
    for b in range(nbatch_total):
        w = 1
        for ls in sorted_lens:
            if b * 128 < len(ls):
                w = max(w, int(ls[b * 128]))
        batches.append(w)
    assert max(batches) <= cfg.TILE_COLS, \
        f"run length {max(batches)} > TILE_COLS"
    placed = []
    t, c = 0, 0
    for k in batches:
        if c + k > cfg.TILE_COLS:
            t += 1
            c = 0
        placed.append((k, t, c))
        c += k
    ntiles = t + 1 if placed else 1
    nbatch = len(placed)
    nchunks = cdiv(nbatch, cfg.NSTAGE)
    chunk_sizes = [min(cfg.NSTAGE, nbatch - i * cfg.NSTAGE) for i in range(nchunks)]
    groups = []
    for bi, (k, t, c0) in enumerate(placed):
        ch = bi // cfg.NSTAGE
        if groups and groups[-1][0] == t and groups[-1][2] == k \
                and groups[-1][5] == ch \
                and groups[-1][1] + groups[-1][2] * groups[-1][3] == c0 \
                and groups[-1][4] + groups[-1][3] == bi:
            t0, c0g, kg, B, b0, chg = groups[-1]
            groups[-1] = (t0, c0g, kg, B + 1, b0, chg)
        else:
            groups.append((t, c0, k, 1, bi, ch))
    return Sched(placed, groups, ntiles, nchunks, nbatch, chunk_sizes)


def _wrap16(flat):
    assert len(flat) % 16 == 0
    b = flat.reshape(-1, 16).T.astype(np.int16)
    return np.tile(b, (8, 1))


SUBMAX = 30


def _subcols(n):
    return [(i, min(SUBMAX, n - i)) for i in range(0, n, SUBMAX)]


def _mk_streams(sched: Sched, starts, lens, gvals, svals, runvals,
                dummy_g, junk_s, dummy_run, cfg: Cfg):
    TC = cfg.TILE_COLS
    g_arr = np.full((sched.ntiles, TC, 128), dummy_g, np.int64)
    s_arr = np.full((sched.nbatch, 128), junk_s, np.int64)
    r_arr = np.full((sched.nbatch, 128), dummy_run, np.int64)
    order = np.argsort(-lens, kind="stable") if len(lens) else np.zeros(0, np.int64)
    for bi, (k, t, c0) in enumerate(sched.batches):
        idxs = order[bi * 128:(bi + 1) * 128]
        nr = len(idxs)
        if nr:
            st = starts[idxs]
            kr = lens[idxs]
            assert kr[0] <= k
            for kk in np.unique(kr):
                sel = np.flatnonzero(kr == kk)
                gm = gvals[st[sel][None, :] + np.arange(kk)[:, None]]
                g_arr[t, c0:c0 + kk, sel] = gm.T
            s_arr[bi, :nr] = svals[idxs]
            r_arr[bi, :nr] = runvals[idxs]
    g_idx = np.concatenate(
        [_wrap16(g_arr[t, c0:c0 + cc].reshape(-1))
         for t in range(sched.ntiles) for (c0, cc) in _subcols(TC)], axis=1)
    sc_blocks = []
    off = 0
    for nb in sched.chunk_sizes:
        sc_blocks.append(_wrap16(s_arr[off:off + nb].reshape(-1)))
        off += nb
    sc_idx = np.concatenate(sc_blocks, axis=1)
    r_idx = np.concatenate(
        [_wrap16(r_arr[b0:b0 + bb].reshape(-1))
         for (b0, bb) in _subcols(sched.nbatch)], axis=1)
    return g_idx, sc_idx, r_idx


def build_plan(node_idx, edge_idx, cfg: Cfg):
    node_idx = np.asarray(node_idx).astype(np.int64)
    edge_idx = np.asarray(edge_idx).astype(np.int64)
    percore = []
    for m in range(cfg.NC):
        sel = np.flatnonzero(node_idx // cfg.NSH == m)
        nl = node_idx[sel] - m * cfg.NSH
        eg = edge_idx[sel]
        sA, lA, vA = _runs(eg)
        oB = np.argsort(nl, kind="stable")
        nB = nl[oB]
        eB = eg[oB]
        sB, lB, vB = _runs(nB)
        percore.append(dict(nl=nl, eg=eg, sA=sA, lA=lA, vA=vA,
                            eB=eB, sB=sB, lB=lB, vB=vB))
    schedA = _mk_schedule([c["lA"] for c in percore], cfg)
    schedB = _mk_schedule([c["lB"] for c in percore], cfg)
    streams = []
    for c in percore:
        gA, scA, _ = _mk_streams(
            schedA, c["sA"], c["lA"],
            gvals=c["nl"], svals=c["vA"], runvals=c["vA"],
            dummy_g=cfg.DUMMY_NODE, junk_s=cfg.JUNK_EDGE,
            dummy_run=cfg.JUNK_EDGE, cfg=cfg)
        gB, scB, rB = _mk_streams(
            schedB, c["sB"], c["lB"],
            gvals=c["eB"], svals=c["vB"], runvals=c["vB"],
            dummy_g=cfg.JUNK_EDGE, junk_s=cfg.DUMMY_NODE,
            dummy_run=cfg.DUMMY_NODE, cfg=cfg)
        streams.append(dict(gA=gA, scA=scA, gB=gB, scB=scB, rB=rB))
    return schedA, schedB, streams


def _ap(t_ap, off, dims):
    base = t_ap
    part = base.ap[0]
    return bass_rust.AP(base.tensor, base.offset + off, [part] + dims)


def build_bass(cfg: Cfg, schedA: Sched, schedB: Sched, replica_groups):
    import os
    _stops = ["init", "phase1", "passA", "coll", "ea", "full"]
    _stop = _stops.index(os.environ.get("GNN_STOP", "full"))
    TC, NS = cfg.TILE_COLS, cfg.NSTAGE
    H, C = cfg.H, cfg.C
    D = cfg.D
    NT, ET = cfg.NT_ROWS, cfg.ET_ROWS
    NSH, E = cfg.NSH, cfg.E
    n_a_node = NT // 128
    n_a_edge = ET // 128

    nc = bacc.Bacc(get_trn_type() or "TRN2", target_bir_lowering=False, debug=False,
                   num_swdge_queues=4)

    # ---- I/O ----
    xT = nc.dram_tensor("xT", [D, NT], F32, kind="ExternalInput")
    haT = nc.dram_tensor("haT", [D, ET], F32, kind="ExternalInput")
    Wn = nc.dram_tensor("Wn", [D, H * C], F32, kind="ExternalInput")
    We = nc.dram_tensor("We", [D, H * C], F32, kind="ExternalInput")
    attn = nc.dram_tensor("attn", [128, H * C], F32, kind="ExternalInput")
    bias_t = nc.dram_tensor("bias_t", [128, 16 * H * C], F32, kind="ExternalInput")
    gA_i = nc.dram_tensor("gA_i", [128, schedA.ntiles * TC * 8], I16, kind="ExternalInput")
    scA_i = nc.dram_tensor("scA_i", [128, schedA.nbatch * 8], I16, kind="ExternalInput")
    gB_i = nc.dram_tensor("gB_i", [128, schedB.ntiles * TC * 8], I16, kind="ExternalInput")
    scB_i = nc.dram_tensor("scB_i", [128, schedB.nbatch * 8], I16, kind="ExternalInput")
    rB_i = nc.dram_tensor("rB_i", [128, schedB.nbatch * 8], I16, kind="ExternalInput")
    y = nc.dram_tensor("y", [NT, H * C], F32, kind="ExternalOutput")
    _dbg = os.environ.get("GNN_DEBUG_OUTS", "0") == "1"
    if _dbg:
        nfw_dbg = nc.dram_tensor("nfw_dbg", [NT, ROW], BF16, kind="ExternalOutput")
        U_dbg = nc.dram_tensor("U_dbg", [ET, ROW], BF16, kind="ExternalOutput")
        Ured_dbg = nc.dram_tensor("Ured_dbg", [ET, ROW], BF16, kind="ExternalOutput")
        EAp_dbg = nc.dram_tensor("EAp_dbg", [ET, 128], BF16, kind="ExternalOutput")

    # ---- internal DRAM ----
    nfw_table = nc.dram_tensor("nfw_table", [NT, ROW], BF16)
    expl_table = nc.dram_tensor("expl_table", [NT, 64], F32)
    ef_table = nc.dram_tensor("ef_table", [ET, 128], BF16)
    U_table = nc.dram_tensor("U_table", [ET, ROW], BF16)
    U_red = nc.dram_tensor("U_red", [ET, ROW], BF16, addr_space="Shared")
    EAp = nc.dram_tensor("EAp", [ET, 128], BF16)

    nfw_v = nfw_table[:].rearrange("(a p) c -> p a c", p=128)
    expl_v = expl_table[:].rearrange("(a p) c -> p a c", p=128)
    ef_v = ef_table[:].rearrange("(a p) c -> p a c", p=128)
    U_v = U_table[:].rearrange("(a p) c -> p a c", p=128)
    Ured_v = U_red[:].rearrange("(a p) c -> p a c", p=128)
    EAp_v = EAp[:].rearrange("(a p) c -> p a c", p=128)
    y_v = y[:].rearrange("(a p) c -> p a c", p=128)

    # Manual DMA-completion tracking for prepare_only SWDGE ops.
    #
    # Tile's DMASW machinery cannot order data-consumers after a prepped
    # DMA's completion: it pre-bumps the lane sem (InstIncSwdgeSem +16)
    # BEFORE the prep, so its consumer waits are satisfied early. We
    # therefore bake our OWN sems into the descriptors (sem=) and attach
    # explicit waits to the first consumer instruction of each prepped
    # transfer via BassInstruction.wait_op. All preps go on queue 0 (the
    # queue Tile's pre-bumps target; one queue still feeds all 16 SDMA
    # engines). Tile's own (early-satisfied) waits remain, harmlessly.
    #
    # Everything ELSE is sound through Tile: a prep defers its table-read /
    # table-write deps to its trigger (tracked against HWDGE/engine sems),
    # and prep-vs-reader buffer WAW gates the prep on tracked engine ticks.
    # Manual additions needed: (1) consumer waits after gather DMAs,
    # (2) WAR on scatter-source staging reuse, (3) scatter-completion
    # gates before the collective / kernel end.

    class _SemCtr:
        def __init__(self, name, n, queues=(0,)):
            self.sems = [nc.alloc_semaphore(f"{name}{i}") for i in range(n)]
            self.vals = [0] * n
            self.queues = [queues[i % len(queues)] for i in range(n)]
            self.i = 0

        def acquire(self):
            s = self.i % len(self.sems)
            self.i += 1
            return s

        def bump(self, slot):
            self.vals[slot] += 16
            return self.sems[slot]

        def cur(self, slot):
            return self.sems[slot], self.vals[slot]

    with tile.TileContext(nc) as tc:
        dma_sems = tc.sems.swdge_block()
        with tc.tile_pool(name="const", bufs=1) as cpool:
            nc.gpsimd.load_library(mlp_library)
            Wn_sb = cpool.tile([D, H * C], F32)
            We_sb = cpool.tile([D, H * C], F32)
            attn_sb = cpool.tile([128, H * C], F32)
            bias_sb = cpool.tile([128, 16 * H * C], F32)
            zbf = cpool.tile([128, 16 * ROW], BF16)
            nc.sync.dma_start(Wn_sb[:], Wn[:])
            nc.sync.dma_start(We_sb[:], We[:])
            nc.sync.dma_start(attn_sb[:], attn[:])
            nc.sync.dma_start(bias_sb[:], bias_t[:])
            nc.gpsimd.memset(zbf[:], 0.0)

            # ---------- init: U=0 (scalar HWDGE queue), y=bias ----------
            zv = zbf[:].rearrange("p (a c) -> p a c", c=ROW)
            for a0 in range(0, n_a_edge, 16):
                aa = min(16, n_a_edge - a0)
                nc.scalar.dma_start(U_v[:, a0:a0 + aa, :], zv[:, :aa, :])
            bv = bias_sb[:].rearrange("p (a c) -> p a c", c=H * C)
            for a0 in range(0, n_a_node, 16):
                aa = min(16, n_a_node - a0)
                nc.scalar.dma_start(y_v[:, a0:a0 + aa, :], bv[:, :aa, :])

            # ---------- phase 1: node projection -> nfw + expl ----------
            if _stop >= 1:
                with (tc.tile_pool(name="p1", bufs=2) as p1,
                    tc.tile_pool(name="p1s", bufs=2) as p1s,
                    tc.tile_pool(name="ps", bufs=4, space="PSUM") as psp):
                  CHK = 16
                  for a0 in range(0, n_a_node, CHK):
                      aa = min(CHK, n_a_node - a0)
                      xc = p1.tile([D, CHK * 128], F32, tag="xc")
                      nc.sync.dma_start(xc[:, :aa * 128], xT[:, a0 * 128:(a0 + aa) * 128])
                      nfst = p1s.tile([128, CHK, 128], F32, tag="nfst")
                      for i in range(aa):
                          mm = psp.tile([128, 128], F32, tag="mm")
                          nc.tensor.matmul(mm[:], xc[:, i * 128:(i + 1) * 128], Wn_sb[:],
                                           start=True, stop=True)
                          nc.vector.tensor_copy(nfst[:, i, :], mm[:])
                      tmp1 = p1s.tile([128, CHK, 128], F32, tag="tmp1")
                      nc.vector.tensor_tensor(
                          out=tmp1[:, :aa, :].rearrange("p a (h c) -> p a h c", h=H),
                          in0=nfst[:, :aa, :].rearrange("p a (h c) -> p a h c", h=H),
                          in1=attn_sb[:].rearrange("p (h c) -> p h c", h=H)
                              .unsqueeze(1).broadcast_to([128, aa, H, C]),
                          op=ALU.mult)
                      praw = p1s.tile([128, CHK, H], F32, tag="praw")
                      nc.vector.tensor_reduce(
                          out=praw[:, :aa, :],
                          in_=tmp1[:, :aa, :].rearrange("p a (h c) -> p a h c", h=H),
                          axis=AX.X, op=ALU.add)
                      est = p1s.tile([128, CHK, 64], F32, tag="est")
                      nc.gpsimd.memset(est[:], 0.0)
                      nc.scalar.activation(est[:, :aa, 0:H], praw[:, :aa, :], ACTF.Exp)
                      nrow = p1s.tile([128, CHK, ROW], BF16, tag="nrow")
                      nc.gpsimd.memset(nrow[:], 0.0)
                      nc.vector.tensor_tensor(
                          out=nrow[:, :aa, 0:128].rearrange("p a (h c) -> p a h c", h=H),
                          in0=nfst[:, :aa, :].rearrange("p a (h c) -> p a h c", h=H),
                          in1=est[:, :aa, 0:H].unsqueeze(3).broadcast_to([128, aa, H, C]),
                          op=ALU.mult)
                      nc.vector.tensor_copy(nrow[:, :aa, 128:UCOL], est[:, :aa, 0:H])
                      nc.sync.dma_start(nfw_v[:, a0:a0 + aa, :], nrow[:, :aa, :])
                      nc.sync.dma_start(expl_v[:, a0:a0 + aa, :], est[:, :aa, :])
                  # zero dummy/pad rows (incl. expl cols -> no D contribution)
                  nc.sync.dma_start(nfw_table[NSH:NT, :], zbf[0:NT - NSH, 0:ROW])

            # ---------- pass A ----------
            if _stop >= 2:
                gsemA = _SemCtr("gA_dma", 6, queues=(1, 2, 3))
                scsemA = _SemCtr("scA_dma", 1)
                # staging bufs=4 makes buffer-reuse WAR sound with NO manual
                # waits: a chunk-ch writer is RAW-gated on reduces, which wait
                # gathers enqueued (queue-0 FIFO) after flush(ch-s-1), s<=3
                # being the max chunks one tile spans; FIFO order then implies
                # scatter(ch-4) has fully drained.
                _span = {}
                for g in schedA.groups:
                    _span.setdefault(g[0], []).append(g[5])
                assert all(max(v) - min(v) <= 2 for v in _span.values()), "tile spans >3 chunks"
                with (tc.tile_pool(name="gA", bufs=2) as gpool,
                    tc.tile_pool(name="stA", bufs=4) as spool,
                    tc.tile_pool(name="idxA", bufs=1) as ipool):
                  gA_sb = ipool.tile([128, schedA.ntiles * TC * 8], I16)
                  scA_sb = ipool.tile([128, schedA.nbatch * 8], I16)
                  nc.sync.dma_start(gA_sb[:], gA_i[:])
                  nc.sync.dma_start(scA_sb[:], scA_i[:])

                  groups_by_tile = {}
                  for g in schedA.groups:
                      groups_by_tile.setdefault(g[0], []).append(g)

                  cur_chunk = [0]
                  stag = {}
                  sc_off = [0]

                  def open_chunk():
                      stag["U"] = spool.tile([128, NS, UCOL], F32, tag="ustag", name="ustag")

                  def flush_chunk():
                      ch = cur_chunk[0]
                      nb = schedA.chunk_sizes[ch]
                      # full 512B rows (scatter payloads must stay on the
                      # proven 256B-multiple geometry); pad cols add zero.
                      # Pads are zeroed once per buffer instance (first 4
                      # chunks) and never dirtied afterwards.
                      ubf = spool.tile([128, NS, ROW], BF16, tag="ubf", name="ubf")
                      if ch < 4:
                          nc.gpsimd.memset(ubf[:, :, UCOL:], 0.0)
                      cast = nc.vector.tensor_copy(ubf[:, :nb, 0:UCOL], stag["U"][:, :nb, :])
                      if ch >= 4:
                          cast.wait_op(scsemA.sems[0], 16 * (ch - 3), "sem-ge")
                      sem = scsemA.bump(0)
                      nc.gpsimd.dma_scatter_add(
                          U_table[:], ubf[:, :nb, :],
                          scA_sb[:, sc_off[0]:sc_off[0] + nb * 8],
                          nb * 128, nb * 128, ROW,
                          prepare_only=True, sem=sem,
                          single_packet=False, queue_num=0)
                      nc.gpsimd.trigger_dma(count=None, queue_num=0)
                      sc_off[0] += nb * 8
                      cur_chunk[0] += 1

                  open_chunk()
                  for t in range(schedA.ntiles):
                      G = gpool.tile([128, TC, ROW], BF16, tag="G")
                      slot = gsemA.acquire()
                      q = gsemA.queues[slot]
                      for (c0s, cc) in _subcols(TC):
                          sem = gsemA.bump(slot)
                          nc.gpsimd.dma_gather(
                              G[:, c0s:c0s + cc, :], nfw_table[:],
                              gA_sb[:, t * TC * 8 + c0s * 8:
                                    t * TC * 8 + (c0s + cc) * 8],
                              cc * 128, cc * 128, ROW,
                              prepare_only=True, sem=sem,
                              single_packet=False, queue_num=q)
                          nc.gpsimd.trigger_dma(count=None, queue_num=q)
                      for (_, c0, k, B, b0, ch) in groups_by_tile.get(t, []):
                          if ch != cur_chunk[0]:
                              flush_chunk()
                              open_chunk()
                          bpos = b0 - ch * NS
                          red = nc.vector.tensor_reduce(
                              out=stag["U"][:, bpos:bpos + B, :],
                              in_=_ap(G[:], c0 * ROW, [[k * ROW, B], [1, UCOL], [ROW, k]]),
                              axis=AX.X, op=ALU.add)
                          # scheduler may reorder same-engine reduces, so
                          # every consumer carries the gather-complete wait
                          s, v = gsemA.cur(slot)
                          red.wait_op(s, v, "sem-ge")
                  flush_chunk()

            # ---------- edge projection (overlaps passA tail / AR) ----------
            if _stop >= 1:
                with (tc.tile_pool(name="pe", bufs=2) as pe,
                    tc.tile_pool(name="pes", bufs=2) as pes,
                    tc.tile_pool(name="ps2", bufs=4, space="PSUM") as psp2):
                  CHK = 16
                  for a0 in range(0, n_a_edge, CHK):
                      aa = min(CHK, n_a_edge - a0)
                      hc = pe.tile([D, CHK * 128], F32, tag="hc")
                      nc.sync.dma_start(hc[:, :aa * 128], haT[:, a0 * 128:(a0 + aa) * 128])
                      efst = pes.tile([128, CHK, 128], BF16, tag="efst")
                      for i in range(aa):
                          mm = psp2.tile([128, 128], F32, tag="mm")
                          nc.tensor.matmul(mm[:], hc[:, i * 128:(i + 1) * 128], We_sb[:],
                                           start=True, stop=True)
                          nc.vector.tensor_copy(efst[:, i, :], mm[:])
                      nc.sync.dma_start(ef_v[:, a0:a0 + aa, :], efst[:, :aa, :])

            # ---------- pass B prologue: index/expl loads (overlap AR) ----------
            if _stop >= 5:
                pb_stack = [tc.tile_pool(name="gB", bufs=2),
                            tc.tile_pool(name="stB", bufs=4),
                            tc.tile_pool(name="idxB", bufs=1)]
                gpoolB, spoolB, ipoolB = [p.__enter__() for p in pb_stack]
                gB_sb = ipoolB.tile([128, schedB.ntiles * TC * 8], I16)
                scB_sb = ipoolB.tile([128, schedB.nbatch * 8], I16)
                rB_sb = ipoolB.tile([128, schedB.nbatch * 8], I16)
                nc.sync.dma_start(gB_sb[:], gB_i[:])
                nc.sync.dma_start(scB_sb[:], scB_i[:])
                nc.sync.dma_start(rB_sb[:], rB_i[:])
                exsem = _SemCtr("explg_dma", 1)
                explg = ipoolB.tile([128, schedB.nbatch, 64], F32)
                for (b0s, bb) in _subcols(schedB.nbatch):
                    sem = exsem.bump(0)
                    nc.gpsimd.dma_gather(
                        explg[:, b0s:b0s + bb, :], expl_table[:],
                        rB_sb[:, b0s * 8:(b0s + bb) * 8],
                        bb * 128, bb * 128, 64,
                        prepare_only=True, sem=sem,
                        single_packet=False, queue_num=0)
                    nc.gpsimd.trigger_dma(count=None, queue_num=0)

            # ---------- collective ----------
            if _stop >= 3:
                coll = nc.gpsimd.collective_compute(
                    "AllReduce", ALU.add, replica_groups=replica_groups,
                    ins=[U_table[:]], outs=[U_red[:]])
                if _stop >= 2:
                    # gate on all pass-A U scatter DMAs having drained
                    s, v = scsemA.cur(0)
                    coll.wait_op(s, v, "sem-ge")

            # ---------- EA' = (U/D + ef)/D ----------
            if _stop >= 4:
                with tc.tile_pool(name="ea", bufs=2) as eap:
                  for a0 in range(0, n_a_edge, 16):
                      aa = min(16, n_a_edge - a0)
                      uc = eap.tile([128, 16, ROW], BF16, tag="uc")
                      efc = eap.tile([128, 16, 128], BF16, tag="efc")
                      nc.sync.dma_start(uc[:, :aa, :], Ured_v[:, a0:a0 + aa, :])
                      nc.sync.dma_start(efc[:, :aa, :], ef_v[:, a0:a0 + aa, :])
                      dv = eap.tile([128, 16, H], F32, tag="dv")
                      nc.vector.tensor_copy(dv[:, :aa, :], uc[:, :aa, 128:UCOL])
                      nc.vector.tensor_scalar_add(dv[:, :aa, :], dv[:, :aa, :], 1e-30)
                      inv = eap.tile([128, 16, H], F32, tag="inv")
                      nc.vector.reciprocal(inv[:, :aa, :], dv[:, :aa, :])
                      inv_b = inv[:, :aa, :].unsqueeze(3).broadcast_to([128, aa, H, C])
                      t1 = eap.tile([128, 16, 128], F32, tag="t1")
                      nc.vector.tensor_tensor(
                          out=t1[:, :aa, :].rearrange("p a (h c) -> p a h c", h=H),
                          in0=uc[:, :aa, 0:128].rearrange("p a (h c) -> p a h c", h=H),
                          in1=inv_b, op=ALU.mult)
                      nc.vector.tensor_tensor(out=t1[:, :aa, :], in0=t1[:, :aa, :],
                                              in1=efc[:, :aa, :], op=ALU.add)
                      eab = eap.tile([128, 16, 128], BF16, tag="eab")
                      nc.vector.tensor_tensor(
                          out=eab[:, :aa, :].rearrange("p a (h c) -> p a h c", h=H),
                          in0=t1[:, :aa, :].rearrange("p a (h c) -> p a h c", h=H),
                          in1=inv_b, op=ALU.mult)
                      nc.sync.dma_start(EAp_v[:, a0:a0 + aa, :], eab[:, :aa, :])
                  nj = ET - E
                  nc.sync.dma_start(EAp[E:ET, :], zbf[0:nj, 0:128])

            if _dbg:
                if _stop >= 1:
                    nc.sync.dma_start(nfw_dbg[:], nfw_table[:])
                if _stop >= 2:
                    d2 = nc.sync.dma_start(U_dbg[:], U_table[:])
                    s, v = scsemA.cur(0)
                    d2.wait_op(s, v, "sem-ge")
                if _stop >= 3:
                    nc.sync.dma_start(Ured_dbg[:], U_red[:])
                if _stop >= 4:
                    nc.sync.dma_start(EAp_dbg[:], EAp[:])

            # ---------- pass B ----------
            if _stop >= 5:
                gsemB = _SemCtr("gB_dma", 6, queues=(1, 2, 3))
                ysem = _SemCtr("scB_dma", 1)
                groups_by_tileB = {}
                for g in schedB.groups:
                    groups_by_tileB.setdefault(g[0], []).append(g)
                cur_chunkB = [0]
                stagB = {}
                sc_offB = [0]
                first_flushB = [True]

                def open_chunkB():
                    stagB["Y"] = spoolB.tile([128, NS, 128], F32, tag="ystag", name="ystag")

                def flush_chunkB():
                    ch = cur_chunkB[0]
                    nb = schedB.chunk_sizes[ch]
                    yst = stagB["Y"]
                    mul = nc.vector.tensor_tensor(
                        out=_ap(yst[:], 0, [[128, nb], [32, H], [1, C]]),
                        in0=_ap(yst[:], 0, [[128, nb], [32, H], [1, C]]),
                        in1=_ap(explg[:], ch * NS * 64, [[64, nb], [1, H], [0, C]]),
                        op=ALU.mult)
                    if first_flushB[0]:
                        s, v = exsem.cur(0)
                        mul.wait_op(s, v, "sem-ge")
                        first_flushB[0] = False
                    sem = ysem.bump(0)
                    nc.gpsimd.dma_scatter_add(
                        y[:], yst[:, :nb, :],
                        scB_sb[:, sc_offB[0]:sc_offB[0] + nb * 8],
                        nb * 128, nb * 128, 128,
                        prepare_only=True, sem=sem,
                        single_packet=False, queue_num=0)
                    nc.gpsimd.trigger_dma(count=None, queue_num=0)
                    sc_offB[0] += nb * 8
                    cur_chunkB[0] += 1

                open_chunkB()
                tile_last_chunk = {}
                for g in schedB.groups:
                    tile_last_chunk[g[0]] = max(tile_last_chunk.get(g[0], 0), g[5])
                for t in range(schedB.ntiles):
                    G = gpoolB.tile([128, TC, 128], BF16, tag="G")
                    slot = gsemB.acquire()
                    q = gsemB.queues[slot]
                    ch_last = tile_last_chunk.get(t, 0)
                    for si, (c0s, cc) in enumerate(_subcols(TC)):
                        sem = gsemB.bump(slot)
                        nc.gpsimd.dma_gather(
                            G[:, c0s:c0s + cc, :], EAp[:],
                            gB_sb[:, t * TC * 8 + c0s * 8:
                                  t * TC * 8 + (c0s + cc) * 8],
                            cc * 128, cc * 128, 128,
                            prepare_only=True, sem=sem,
                            single_packet=False, queue_num=q)
                        trig = nc.gpsimd.trigger_dma(count=None, queue_num=q)
                        if si == 0 and ch_last >= 4:
                            # WAR for yst (bufs=4): gate the tile's gather
                            # DMAs on chunk ch_last-4's y-scatter having
                            # drained. Trigger-side wait is deadlock-safe:
                            # the awaited DMA's doorbell was already rung
                            # by an earlier in-order Pool trigger.
                            trig.wait_op(ysem.sems[0], 16 * (ch_last - 3), "sem-ge")
                    for (_, c0, k, B, b0, ch) in groups_by_tileB.get(t, []):
                        if ch != cur_chunkB[0]:
                            flush_chunkB()
                            open_chunkB()
                        bpos = b0 - ch * NS
                        red = nc.vector.tensor_reduce(
                            out=stagB["Y"][:, bpos:bpos + B, :],
                            in_=_ap(G[:], c0 * 128, [[k * 128, B], [1, 128], [128, k]]),
                            axis=AX.X, op=ALU.add)
                        s, v = gsemB.cur(slot)
                        red.wait_op(s, v, "sem-ge")
                flush_chunkB()
                # terminal guard: kernel end must not race the last y
                # scatter DMAs; hang the wait on a real consumer of yst.
                guard = spoolB.tile([128, 1, 128], F32, tag="guard", name="guard")
                g = nc.vector.tensor_copy(guard[:, 0, :], stagB["Y"][:, 0, :])
                s, v = ysem.cur(0)
                g.wait_op(s, v, "sem-ge")
                for p in reversed(pb_stack):
                    p.__exit__(None, None, None)
    nc.compile()
    return nc


def host_inputs(cfg: Cfg, x, ha, W_node, W_edge, attn_l, bias, streams):
    x = np.asarray(x, np.float32)
    ha = np.asarray(ha, np.float32)
    W_node = np.asarray(W_node, np.float32)
    W_edge = np.asarray(W_edge, np.float32)
    attn_flat = np.asarray(attn_l, np.float32).reshape(-1)
    bias = np.asarray(bias, np.float32).reshape(-1)
    attn_rep = np.tile(attn_flat[None, :], (128, 1))
    bias_t = np.tile(bias[None, :], (128, 16))
    ha_pad = np.zeros((cfg.ET_ROWS, cfg.D), np.float32)
    ha_pad[:cfg.E] = ha
    haT = np.ascontiguousarray(ha_pad.T)
    in_maps = []
    for m in range(cfg.NC):
        xs = np.zeros((cfg.NT_ROWS, cfg.D), np.float32)
        xs[:cfg.NSH] = x[m * cfg.NSH:(m + 1) * cfg.NSH]
        st = streams[m]
        in_maps.append({
            "xT": np.ascontiguousarray(xs.T),
            "haT": haT,
            "Wn": W_node, "We": W_edge,
            "attn": attn_rep, "bias_t": bias_t,
            "gA_i": st["gA"], "scA_i": st["scA"],
            "gB_i": st["gB"], "scB_i": st["scB"], "rB_i": st["rB"],
        })
    return in_maps


# ======================== public entry point ========================
_CFG = Cfg()
LAST_RESULTS = None


def _install_axon_ntff_shim():
    import sys, types, ctypes, contextlib
    import concourse.bass_utils as bu
    bu.upload_artifacts = lambda d: str(d)
    try:
        from antenv.axon_hooks import get_axon_ntff_profile_hook  # noqa
        return
    except ImportError:
        pass
    so_path = "/opt/axon/libaxon_pjrt.so"
    try:
        lib = ctypes.CDLL(so_path)
    except OSError:
        return
    if not hasattr(lib, "axon_start_nrt_profile"):
        return
    lib.axon_start_nrt_profile.argtypes = [ctypes.POINTER(ctypes.c_int64),
                                           ctypes.c_size_t]
    lib.axon_start_nrt_profile.restype = ctypes.c_int64
    lib.axon_stop_nrt_profile.argtypes = [ctypes.c_char_p]
    lib.axon_stop_nrt_profile.restype = ctypes.c_int64

    @contextlib.contextmanager
    def _hook(output_dir, device_ids):
        import jax
        jax.devices()
        if device_ids:
            ids = (ctypes.c_int64 * len(device_ids))(*device_ids)
            rc = lib.axon_start_nrt_profile(ids, len(device_ids))
        else:
            rc = lib.axon_start_nrt_profile(None, 0)
        if rc != 0:
            raise RuntimeError(f"axon_start_nrt_profile rc={rc}")
        try:
            yield
        finally:
            n = lib.axon_stop_nrt_profile(str(output_dir).encode())
            print(f"ntff profile: {n} file(s) -> {output_dir}")

    mod = types.ModuleType("antenv.axon_hooks")
    mod.get_axon_ntff_profile_hook = lambda: _hook
    mod.set_axon_ntff_profile_hook = lambda h: None
    sys.modules["antenv.axon_hooks"] = mod


def kernel(**inputs) -> np.ndarray:
    import os
    from concourse.bass_utils import run_bass_kernel_spmd
    cfg = _CFG
    x = np.asarray(inputs["x"], np.float32)
    ha = np.asarray(inputs["hyperedge_attr"], np.float32)
    node_idx = np.asarray(inputs["node_idx"]).astype(np.int64)
    edge_idx = np.asarray(inputs["edge_idx"]).astype(np.int64)
    schedA, schedB, streams = build_plan(node_idx, edge_idx, cfg)
    nc = build_bass(cfg, schedA, schedB, [list(range(cfg.NC))])
    in_maps = host_inputs(cfg, x, ha, inputs["W_node"], inputs["W_edge"],
                          inputs["attn_l"], inputs["bias"], streams)
    trace = os.environ.get("GNN_TRACE", "0") == "1"
    if trace:
        _install_axon_ntff_shim()
    res = run_bass_kernel_spmd(nc, in_maps, list(range(cfg.NC)), trace=trace)
    global LAST_RESULTS
    LAST_RESULTS = res
    out = np.concatenate(
        [np.asarray(res.results[m]["y"])[:cfg.NSH] for m in range(cfg.NC)], axis=0)
    return np.ascontiguousarray(out, dtype=np.float32)


# revision 30
# speedup vs baseline: 1.1165x; 1.0033x over previous
"""Hypergraph conv kernel, v2.

Pipeline (node-sharded, 8 cores):
  phase1: nfw_table[n] = bf16([exp(a_n)*nf_n (128) | exp(a_n) (4) | pad]),
          expl_table[n] = f32 exp(a_n) (for pass B).
  passA:  per edge-run batch: gather nfw rows, ONE reduce -> [U|D] partial,
          cast bf16, scatter-add into U_table[ET,256] (cols 0:132).
  AR:     one bf16 AllReduce of U_table.
  EA:     EAp[e] = bf16((U/D + ef)/D)   (ef projected during passA window)
  passB:  gather EAp rows per incidence, ONE reduce per run group,
          multiply by expl per chunk, scatter-add into y.

All SWDGE gathers/scatters use prepare_only + trigger_dma so descriptor
generation on the Pool engine decouples from the DMA transfers (the v1
bottleneck: GpSimd 75% busy holding each gather ~6.4us while DMA sat 40%).
DMA-completion sems round-robin over 8 handles, mirroring Tile's DMASW
lane rotation (preps are the only Pool-engine DMA instructions emitted).
"""
import numpy as np
from dataclasses import dataclass

import concourse.bass as bass
import concourse.mybir as mybir
import concourse.bacc as bacc
import concourse.tile as tile
import bass_rust
from concourse.library_config import mlp as mlp_library
from concourse._compat import get_trn_type, cdiv

F32 = mybir.dt.float32
BF16 = mybir.dt.bfloat16
I16 = mybir.dt.int16
AX = mybir.AxisListType
ALU = mybir.AluOpType
ACTF = mybir.ActivationFunctionType

ROW = 256          # nfw/U table row width (bf16 elems); cols 0:128 nfw, 128:132 expl
UCOL = 132         # useful cols in nfw/U rows


@dataclass
class Cfg:
    N: int = 100000
    E: int = 25000
    D: int = 128
    H: int = 4
    C: int = 32
    NC: int = 8
    TILE_COLS: int = 30
    NSTAGE: int = 14

    @property
    def NSH(self):
        return self.N // self.NC

    @property
    def NT_ROWS(self):
        return cdiv(self.NSH + 1, 128) * 128

    @property
    def ET_ROWS(self):
        return cdiv(self.E + 1, 128) * 128

    @property
    def DUMMY_NODE(self):
        return self.NSH

    @property
    def JUNK_EDGE(self):
        return self.E


def _runs(keys):
    if len(keys) == 0:
        return (np.zeros(0, np.int64),) * 3
    change = np.flatnonzero(np.diff(keys)) + 1
    starts = np.concatenate([[0], change]).astype(np.int64)
    ends = np.concatenate([change, [len(keys)]]).astype(np.int64)
    return starts, ends - starts, keys[starts].astype(np.int64)


@dataclass
class Sched:
    batches: list          # [(k, tile, c0)]
    groups: list           # [(tile, c0, k, B, b0, chunk)]
    ntiles: int
    nchunks: int
    nbatch: int
    chunk_sizes: list


def _mk_schedule(lens_list, cfg: Cfg) -> Sched:
    sorted_lens = [np.sort(np.asarray(l))[::-1] for l in lens_list]
    nbatch_total = max(cdiv(len(l), 128) for l in sorted_lens)
    batches = []
    for b in range(nbatch_total):
        w = 1
        for ls in sorted_lens:
            if b * 128 < len(ls):
                w = max(w, int(ls[b * 128]))
        batches.append(w)
    assert max(batches) <= cfg.TILE_COLS, \
        f"run length {max(batches)} > TILE_COLS"
    placed = []
    t, c = 0, 0
    for k in batches:
        if c + k > cfg.TILE_COLS:
            t += 1
            c = 0
        placed.append((k, t, c))
        c += k
    ntiles = t + 1 if placed else 1
    nbatch = len(placed)
    nchunks = cdiv(nbatch, cfg.NSTAGE)
    chunk_sizes = [min(cfg.NSTAGE, nbatch - i * cfg.NSTAGE) for i in range(nchunks)]
    groups = []
    for bi, (k, t, c0) in enumerate(placed):
        ch = bi // cfg.NSTAGE
        if groups and groups[-1][0] == t and groups[-1][2] == k \
                and groups[-1][5] == ch \
                and groups[-1][1] + groups[-1][2] * groups[-1][3] == c0 \
                and groups[-1][4] + groups[-1][3] == bi:
            t0, c0g, kg, B, b0, chg = groups[-1]
            groups[-1] = (t0, c0g, kg, B + 1, b0, chg)
        else:
            groups.append((t, c0, k, 1, bi, ch))
    return Sched(placed, groups, ntiles, nchunks, nbatch, chunk_sizes)


def _wrap16(flat):
    assert len(flat) % 16 == 0
    b = flat.reshape(-1, 16).T.astype(np.int16)
    return np.tile(b, (8, 1))


SUBMAX = 30


def _subcols(n):
    return [(i, min(SUBMAX, n - i)) for i in range(0, n, SUBMAX)]


def _mk_streams(sched: Sched, starts, lens, gvals, svals, runvals,
                dummy_g, junk_s, dummy_run, cfg: Cfg):
    TC = cfg.TILE_COLS
    g_arr = np.full((sched.ntiles, TC, 128), dummy_g, np.int64)
    s_arr = np.full((sched.nbatch, 128), junk_s, np.int64)
    r_arr = np.full((sched.nbatch, 128), dummy_run, np.int64)
    order = np.argsort(-lens, kind="stable") if len(lens) else np.zeros(0, np.int64)
    for bi, (k, t, c0) in enumerate(sched.batches):
        idxs = order[bi * 128:(bi + 1) * 128]
        nr = len(idxs)
        if nr:
            st = starts[idxs]
            kr = lens[idxs]
            assert kr[0] <= k
            for kk in np.unique(kr):
                sel = np.flatnonzero(kr == kk)
                gm = gvals[st[sel][None, :] + np.arange(kk)[:, None]]
                g_arr[t, c0:c0 + kk, sel] = gm.T
            s_arr[bi, :nr] = svals[idxs]
            r_arr[bi, :nr] = runvals[idxs]
    g_idx = np.concatenate(
        [_wrap16(g_arr[t, c0:c0 + cc].reshape(-1))
         for t in range(sched.ntiles) for (c0, cc) in _subcols(TC)], axis=1)
    sc_blocks = []
    off = 0
    for nb in sched.chunk_sizes:
        sc_blocks.append(_wrap16(s_arr[off:off + nb].reshape(-1)))
        off += nb
    sc_idx = np.concatenate(sc_blocks, axis=1)
    r_idx = np.concatenate(
        [_wrap16(r_arr[b0:b0 + bb].reshape(-1))
         for (b0, bb) in _subcols(sched.nbatch)], axis=1)
    return g_idx, sc_idx, r_idx


def build_plan(node_idx, edge_idx, cfg: Cfg):
    node_idx = np.asarray(node_idx).astype(np.int64)
    edge_idx = np.asarray(edge_idx).astype(np.int64)
    percore = []
    for m in range(cfg.NC):
        sel = np.flatnonzero(node_idx // cfg.NSH == m)
        nl = node_idx[sel] - m * cfg.NSH
        eg = edge_idx[sel]
        sA, lA, vA = _runs(eg)
        oB = np.argsort(nl, kind="stable")
        nB = nl[oB]
        eB = eg[oB]
        sB, lB, vB = _runs(nB)
        percore.append(dict(nl=nl, eg=eg, sA=sA, lA=lA, vA=vA,
                            eB=eB, sB=sB, lB=lB, vB=vB))
    schedA = _mk_schedule([c["lA"] for c in percore], cfg)
    schedB = _mk_schedule([c["lB"] for c in percore], cfg)
    streams = []
    for c in percore:
        gA, scA, _ = _mk_streams(
            schedA, c["sA"], c["lA"],
            gvals=c["nl"], svals=c["vA"], runvals=c["vA"],
            dummy_g=cfg.DUMMY_NODE, junk_s=cfg.JUNK_EDGE,
            dummy_run=cfg.JUNK_EDGE, cfg=cfg)
        gB, scB, rB = _mk_streams(
            schedB, c["sB"], c["lB"],
            gvals=c["eB"], svals=c["vB"], runvals=c["vB"],
            dummy_g=cfg.JUNK_EDGE, junk_s=cfg.DUMMY_NODE,
            dummy_run=cfg.DUMMY_NODE, cfg=cfg)
        streams.append(dict(gA=gA, scA=scA, gB=gB, scB=scB, rB=rB))
    return schedA, schedB, streams


def _ap(t_ap, off, dims):
    base = t_ap
    part = base.ap[0]
    return bass_rust.AP(base.tensor, base.offset + off, [part] + dims)


def build_bass(cfg: Cfg, schedA: Sched, schedB: Sched, replica_groups):
    import os
    _stops = ["init", "phase1", "passA", "coll", "ea", "full"]
    _stop = _stops.index(os.environ.get("GNN_STOP", "full"))
    TC, NS = cfg.TILE_COLS, cfg.NSTAGE
    H, C = cfg.H, cfg.C
    D = cfg.D
    NT, ET = cfg.NT_ROWS, cfg.ET_ROWS
    NSH, E = cfg.NSH, cfg.E
    n_a_node = NT // 128
    n_a_edge = ET // 128

    nc = bacc.Bacc(get_trn_type() or "TRN2", target_bir_lowering=False, debug=False,
                   num_swdge_queues=4)

    # ---- I/O ----
    xT = nc.dram_tensor("xT", [D, NT], F32, kind="ExternalInput")
    haT = nc.dram_tensor("haT", [D, ET], F32, kind="ExternalInput")
    Wn = nc.dram_tensor("Wn", [D, H * C], F32, kind="ExternalInput")
    We = nc.dram_tensor("We", [D, H * C], F32, kind="ExternalInput")
    attn = nc.dram_tensor("attn", [128, H * C], F32, kind="ExternalInput")
    bias_t = nc.dram_tensor("bias_t", [128, 16 * H * C], F32, kind="ExternalInput")
    gA_i = nc.dram_tensor("gA_i", [128, schedA.ntiles * TC * 8], I16, kind="ExternalInput")
    scA_i = nc.dram_tensor("scA_i", [128, schedA.nbatch * 8], I16, kind="ExternalInput")
    gB_i = nc.dram_tensor("gB_i", [128, schedB.ntiles * TC * 8], I16, kind="ExternalInput")
    scB_i = nc.dram_tensor("scB_i", [128, schedB.nbatch * 8], I16, kind="ExternalInput")
    rB_i = nc.dram_tensor("rB_i", [128, schedB.nbatch * 8], I16, kind="ExternalInput")
    y = nc.dram_tensor("y", [NT, H * C], F32, kind="ExternalOutput")
    _dbg = os.environ.get("GNN_DEBUG_OUTS", "0") == "1"
    if _dbg:
        nfw_dbg = nc.dram_tensor("nfw_dbg", [NT, ROW], BF16, kind="ExternalOutput")
        U_dbg = nc.dram_tensor("U_dbg", [ET, ROW], BF16, kind="ExternalOutput")
        Ured_dbg = nc.dram_tensor("Ured_dbg", [ET, ROW], BF16, kind="ExternalOutput")
        EAp_dbg = nc.dram_tensor("EAp_dbg", [ET, 128], BF16, kind="ExternalOutput")

    # ---- internal DRAM ----
    nfw_table = nc.dram_tensor("nfw_table", [NT, ROW], BF16)
    expl_table = nc.dram_tensor("expl_table", [NT, 64], F32)
    ef_table = nc.dram_tensor("ef_table", [ET, 128], BF16)
    U_table = nc.dram_tensor("U_table", [ET, ROW], BF16)
    U_red = nc.dram_tensor("U_red", [ET, ROW], BF16, addr_space="Shared")
    EAp = nc.dram_tensor("EAp", [ET, 128], BF16)

    nfw_v = nfw_table[:].rearrange("(a p) c -> p a c", p=128)
    expl_v = expl_table[:].rearrange("(a p) c -> p a c", p=128)
    ef_v = ef_table[:].rearrange("(a p) c -> p a c", p=128)
    U_v = U_table[:].rearrange("(a p) c -> p a c", p=128)
    Ured_v = U_red[:].rearrange("(a p) c -> p a c", p=128)
    EAp_v = EAp[:].rearrange("(a p) c -> p a c", p=128)
    y_v = y[:].rearrange("(a p) c -> p a c", p=128)

    # Manual DMA-completion tracking for prepare_only SWDGE ops.
    #
    # Tile's DMASW machinery cannot order data-consumers after a prepped
    # DMA's completion: it pre-bumps the lane sem (InstIncSwdgeSem +16)
    # BEFORE the prep, so its consumer waits are satisfied early. We
    # therefore bake our OWN sems into the descriptors (sem=) and attach
    # explicit waits to the first consumer instruction of each prepped
    # transfer via BassInstruction.wait_op. All preps go on queue 0 (the
    # queue Tile's pre-bumps target; one queue still feeds all 16 SDMA
    # engines). Tile's own (early-satisfied) waits remain, harmlessly.
    #
    # Everything ELSE is sound through Tile: a prep defers its table-read /
    # table-write deps to its trigger (tracked against HWDGE/engine sems),
    # and prep-vs-reader buffer WAW gates the prep on tracked engine ticks.
    # Manual additions needed: (1) consumer waits after gather DMAs,
    # (2) WAR on scatter-source staging reuse, (3) scatter-completion
    # gates before the collective / kernel end.

    class _SemCtr:
        def __init__(self, name, n, queues=(0,)):
            self.sems = [nc.alloc_semaphore(f"{name}{i}") for i in range(n)]
            self.vals = [0] * n
            self.queues = [queues[i % len(queues)] for i in range(n)]
            self.i = 0

        def acquire(self):
            s = self.i % len(self.sems)
            self.i += 1
            return s

        def bump(self, slot):
            self.vals[slot] += 16
            return self.sems[slot]

        def cur(self, slot):
            return self.sems[slot], self.vals[slot]

    with tile.TileContext(nc) as tc:
        dma_sems = tc.sems.swdge_block()
        with tc.tile_pool(name="const", bufs=1) as cpool:
            nc.gpsimd.load_library(mlp_library)
            Wn_sb = cpool.tile([D, H * C], F32)
            We_sb = cpool.tile([D, H * C], F32)
            attn_sb = cpool.tile([128, H * C], F32)
            bias_sb = cpool.tile([128, 16 * H * C], F32)
            zbf = cpool.tile([128, 16 * ROW], BF16)
            nc.sync.dma_start(Wn_sb[:], Wn[:])
            nc.sync.dma_start(We_sb[:], We[:])
            nc.sync.dma_start(attn_sb[:], attn[:])
            nc.sync.dma_start(bias_sb[:], bias_t[:])
            nc.gpsimd.memset(zbf[:], 0.0)

            # ---------- init: U=0 (scalar HWDGE queue), y=bias ----------
            zv = zbf[:].rearrange("p (a c) -> p a c", c=ROW)
            for a0 in range(0, n_a_edge, 16):
                aa = min(16, n_a_edge - a0)
                nc.scalar.dma_start(U_v[:, a0:a0 + aa, :], zv[:, :aa, :])
            bv = bias_sb[:].rearrange("p (a c) -> p a c", c=H * C)
            for a0 in range(0, n_a_node, 16):
                aa = min(16, n_a_node - a0)
                nc.scalar.dma_start(y_v[:, a0:a0 + aa, :], bv[:, :aa, :])

            # ---------- phase 1: node projection -> nfw + expl ----------
            if _stop >= 1:
                with (tc.tile_pool(name="p1", bufs=2) as p1,
                    tc.tile_pool(name="p1s", bufs=2) as p1s,
                    tc.tile_pool(name="ps", bufs=4, space="PSUM") as psp):
                  CHK = 16
                  for a0 in range(0, n_a_node, CHK):
                      aa = min(CHK, n_a_node - a0)
                      xc = p1.tile([D, CHK * 128], F32, tag="xc")
                      nc.sync.dma_start(xc[:, :aa * 128], xT[:, a0 * 128:(a0 + aa) * 128])
                      nfst = p1s.tile([128, CHK, 128], F32, tag="nfst")
                      for i in range(aa):
                          mm = psp.tile([128, 128], F32, tag="mm")
                          nc.tensor.matmul(mm[:], xc[:, i * 128:(i + 1) * 128], Wn_sb[:],
                                           start=True, stop=True)
                          nc.vector.tensor_copy(nfst[:, i, :], mm[:])
                      tmp1 = p1s.tile([128, CHK, 128], F32, tag="tmp1")
                      nc.vector.tensor_tensor(
                          out=tmp1[:, :aa, :].rearrange("p a (h c) -> p a h c", h=H),
                          in0=nfst[:, :aa, :].rearrange("p a (h c) -> p a h c", h=H),
                          in1=attn_sb[:].rearrange("p (h c) -> p h c", h=H)
                              .unsqueeze(1).broadcast_to([128, aa, H, C]),
                          op=ALU.mult)
                      praw = p1s.tile([128, CHK, H], F32, tag="praw")
                      nc.vector.tensor_reduce(
                          out=praw[:, :aa, :],
                          in_=tmp1[:, :aa, :].rearrange("p a (h c) -> p a h c", h=H),
                          axis=AX.X, op=ALU.add)
                      est = p1s.tile([128, CHK, 64], F32, tag="est")
                      nc.gpsimd.memset(est[:], 0.0)
                      nc.scalar.activation(est[:, :aa, 0:H], praw[:, :aa, :], ACTF.Exp)
                      nrow = p1s.tile([128, CHK, ROW], BF16, tag="nrow")
                      nc.gpsimd.memset(nrow[:], 0.0)
                      nc.vector.tensor_tensor(
                          out=nrow[:, :aa, 0:128].rearrange("p a (h c) -> p a h c", h=H),
                          in0=nfst[:, :aa, :].rearrange("p a (h c) -> p a h c", h=H),
                          in1=est[:, :aa, 0:H].unsqueeze(3).broadcast_to([128, aa, H, C]),
                          op=ALU.mult)
                      nc.vector.tensor_copy(nrow[:, :aa, 128:UCOL], est[:, :aa, 0:H])
                      nc.sync.dma_start(nfw_v[:, a0:a0 + aa, :], nrow[:, :aa, :])
                      nc.sync.dma_start(expl_v[:, a0:a0 + aa, :], est[:, :aa, :])
                  # zero dummy/pad rows (incl. expl cols -> no D contribution)
                  nc.sync.dma_start(nfw_table[NSH:NT, :], zbf[0:NT - NSH, 0:ROW])

            # ---------- pass A ----------
            if _stop >= 2:
                gsemA = _SemCtr("gA_dma", 6, queues=(1, 2, 3))
                scsemA = _SemCtr("scA_dma", 1)
                # staging bufs=4 makes buffer-reuse WAR sound with NO manual
                # waits: a chunk-ch writer is RAW-gated on reduces, which wait
                # gathers enqueued (queue-0 FIFO) after flush(ch-s-1), s<=3
                # being the max chunks one tile spans; FIFO order then implies
                # scatter(ch-4) has fully drained.
                _span = {}
                for g in schedA.groups:
                    _span.setdefault(g[0], []).append(g[5])
                assert all(max(v) - min(v) <= 2 for v in _span.values()), "tile spans >3 chunks"
                with (tc.tile_pool(name="gA", bufs=4) as gpool,
                    tc.tile_pool(name="stA", bufs=4) as spool,
                    tc.tile_pool(name="idxA", bufs=1) as ipool):
                  gA_sb = ipool.tile([128, schedA.ntiles * TC * 8], I16)
                  scA_sb = ipool.tile([128, schedA.nbatch * 8], I16)
                  nc.sync.dma_start(gA_sb[:], gA_i[:])
                  nc.sync.dma_start(scA_sb[:], scA_i[:])

                  groups_by_tile = {}
                  for g in schedA.groups:
                      groups_by_tile.setdefault(g[0], []).append(g)

                  cur_chunk = [0]
                  stag = {}
                  sc_off = [0]

                  def open_chunk():
                      stag["U"] = spool.tile([128, NS, UCOL], F32, tag="ustag", name="ustag")

                  def flush_chunk():
                      ch = cur_chunk[0]
                      nb = schedA.chunk_sizes[ch]
                      # full 512B rows (scatter payloads must stay on the
                      # proven 256B-multiple geometry); pad cols add zero.
                      # Pads are zeroed once per buffer instance (first 4
                      # chunks) and never dirtied afterwards.
                      ubf = spool.tile([128, NS, ROW], BF16, tag="ubf", name="ubf")
                      if ch < 4:
                          nc.gpsimd.memset(ubf[:, :, UCOL:], 0.0)
                      cast = nc.vector.tensor_copy(ubf[:, :nb, 0:UCOL], stag["U"][:, :nb, :])
                      if ch >= 4:
                          cast.wait_op(scsemA.sems[0], 16 * (ch - 3), "sem-ge")
                      sem = scsemA.bump(0)
                      nc.gpsimd.dma_scatter_add(
                          U_table[:], ubf[:, :nb, :],
                          scA_sb[:, sc_off[0]:sc_off[0] + nb * 8],
                          nb * 128, nb * 128, ROW,
                          prepare_only=True, sem=sem,
                          single_packet=False, queue_num=0)
                      nc.gpsimd.trigger_dma(count=None, queue_num=0)
                      sc_off[0] += nb * 8
                      cur_chunk[0] += 1

                  open_chunk()
                  for t in range(schedA.ntiles):
                      G = gpool.tile([128, TC, ROW], BF16, tag="G")
                      slot = gsemA.acquire()
                      q = gsemA.queues[slot]
                      for (c0s, cc) in _subcols(TC):
                          sem = gsemA.bump(slot)
                          nc.gpsimd.dma_gather(
                              G[:, c0s:c0s + cc, :], nfw_table[:],
                              gA_sb[:, t * TC * 8 + c0s * 8:
                                    t * TC * 8 + (c0s + cc) * 8],
                              cc * 128, cc * 128, ROW,
                              prepare_only=True, sem=sem,
                              single_packet=False, queue_num=q)
                          nc.gpsimd.trigger_dma(count=None, queue_num=q)
                      for (_, c0, k, B, b0, ch) in groups_by_tile.get(t, []):
                          if ch != cur_chunk[0]:
                              flush_chunk()
                              open_chunk()
                          bpos = b0 - ch * NS
                          red = nc.vector.tensor_reduce(
                              out=stag["U"][:, bpos:bpos + B, :],
                              in_=_ap(G[:], c0 * ROW, [[k * ROW, B], [1, UCOL], [ROW, k]]),
                              axis=AX.X, op=ALU.add)
                          # scheduler may reorder same-engine reduces, so
                          # every consumer carries the gather-complete wait
                          s, v = gsemA.cur(slot)
                          red.wait_op(s, v, "sem-ge")
                  flush_chunk()

            # ---------- edge projection (overlaps passA tail / AR) ----------
            if _stop >= 1:
                with (tc.tile_pool(name="pe", bufs=2) as pe,
                    tc.tile_pool(name="pes", bufs=2) as pes,
                    tc.tile_pool(name="ps2", bufs=4, space="PSUM") as psp2):
                  CHK = 16
                  for a0 in range(0, n_a_edge, CHK):
                      aa = min(CHK, n_a_edge - a0)
                      hc = pe.tile([D, CHK * 128], F32, tag="hc")
                      nc.sync.dma_start(hc[:, :aa * 128], haT[:, a0 * 128:(a0 + aa) * 128])
                      efst = pes.tile([128, CHK, 128], BF16, tag="efst")
                      for i in range(aa):
                          mm = psp2.tile([128, 128], F32, tag="mm")
                          nc.tensor.matmul(mm[:], hc[:, i * 128:(i + 1) * 128], We_sb[:],
                                           start=True, stop=True)
                          nc.vector.tensor_copy(efst[:, i, :], mm[:])
                      nc.sync.dma_start(ef_v[:, a0:a0 + aa, :], efst[:, :aa, :])

            # ---------- pass B prologue: index/expl loads (overlap AR) ----------
            if _stop >= 5:
                pb_stack = [tc.tile_pool(name="gB", bufs=6),
                            tc.tile_pool(name="stB", bufs=4),
                            tc.tile_pool(name="idxB", bufs=1)]
                gpoolB, spoolB, ipoolB = [p.__enter__() for p in pb_stack]
                gB_sb = ipoolB.tile([128, schedB.ntiles * TC * 8], I16)
                scB_sb = ipoolB.tile([128, schedB.nbatch * 8], I16)
                rB_sb = ipoolB.tile([128, schedB.nbatch * 8], I16)
                nc.sync.dma_start(gB_sb[:], gB_i[:])
                nc.sync.dma_start(scB_sb[:], scB_i[:])
                nc.sync.dma_start(rB_sb[:], rB_i[:])
                exsem = _SemCtr("explg_dma", 1)
                explg = ipoolB.tile([128, schedB.nbatch, 64], F32)
                for (b0s, bb) in _subcols(schedB.nbatch):
                    sem = exsem.bump(0)
                    nc.gpsimd.dma_gather(
                        explg[:, b0s:b0s + bb, :], expl_table[:],
                        rB_sb[:, b0s * 8:(b0s + bb) * 8],
                        bb * 128, bb * 128, 64,
                        prepare_only=True, sem=sem,
                        single_packet=False, queue_num=0)
                    nc.gpsimd.trigger_dma(count=None, queue_num=0)

            # ---------- collective ----------
            if _stop >= 3:
                coll = nc.gpsimd.collective_compute(
                    "AllReduce", ALU.add, replica_groups=replica_groups,
                    ins=[U_table[:]], outs=[U_red[:]])
                if _stop >= 2:
                    # gate on all pass-A U scatter DMAs having drained
                    s, v = scsemA.cur(0)
                    coll.wait_op(s, v, "sem-ge")

            # ---------- EA' = (U/D + ef)/D ----------
            if _stop >= 4:
                with tc.tile_pool(name="ea", bufs=2) as eap:
                  for a0 in range(0, n_a_edge, 16):
                      aa = min(16, n_a_edge - a0)
                      uc = eap.tile([128, 16, ROW], BF16, tag="uc")
                      efc = eap.tile([128, 16, 128], BF16, tag="efc")
                      nc.sync.dma_start(uc[:, :aa, :], Ured_v[:, a0:a0 + aa, :])
                      nc.sync.dma_start(efc[:, :aa, :], ef_v[:, a0:a0 + aa, :])
                      dv = eap.tile([128, 16, H], F32, tag="dv")
                      nc.vector.tensor_copy(dv[:, :aa, :], uc[:, :aa, 128:UCOL])
                      nc.vector.tensor_scalar_add(dv[:, :aa, :], dv[:, :aa, :], 1e-30)
                      inv = eap.tile([128, 16, H], F32, tag="inv")
                      nc.vector.reciprocal(inv[:, :aa, :], dv[:, :aa, :])
                      inv_b = inv[:, :aa, :].unsqueeze(3).broadcast_to([128, aa, H, C])
                      t1 = eap.tile([128, 16, 128], F32, tag="t1")
                      nc.vector.tensor_tensor(
                          out=t1[:, :aa, :].rearrange("p a (h c) -> p a h c", h=H),
                          in0=uc[:, :aa, 0:128].rearrange("p a (h c) -> p a h c", h=H),
                          in1=inv_b, op=ALU.mult)
                      nc.vector.tensor_tensor(out=t1[:, :aa, :], in0=t1[:, :aa, :],
                                              in1=efc[:, :aa, :], op=ALU.add)
                      eab = eap.tile([128, 16, 128], BF16, tag="eab")
                      nc.vector.tensor_tensor(
                          out=eab[:, :aa, :].rearrange("p a (h c) -> p a h c", h=H),
                          in0=t1[:, :aa, :].rearrange("p a (h c) -> p a h c", h=H),
                          in1=inv_b, op=ALU.mult)
                      nc.sync.dma_start(EAp_v[:, a0:a0 + aa, :], eab[:, :aa, :])
                  nj = ET - E
                  nc.sync.dma_start(EAp[E:ET, :], zbf[0:nj, 0:128])

            if _dbg:
                if _stop >= 1:
                    nc.sync.dma_start(nfw_dbg[:], nfw_table[:])
                if _stop >= 2:
                    d2 = nc.sync.dma_start(U_dbg[:], U_table[:])
                    s, v = scsemA.cur(0)
                    d2.wait_op(s, v, "sem-ge")
                if _stop >= 3:
                    nc.sync.dma_start(Ured_dbg[:], U_red[:])
                if _stop >= 4:
                    nc.sync.dma_start(EAp_dbg[:], EAp[:])

            # ---------- pass B ----------
            if _stop >= 5:
                gsemB = _SemCtr("gB_dma", 6, queues=(1, 2, 3))
                ysem = _SemCtr("scB_dma", 1)
                groups_by_tileB = {}
                for g in schedB.groups:
                    groups_by_tileB.setdefault(g[0], []).append(g)
                cur_chunkB = [0]
                stagB = {}
                sc_offB = [0]
                first_flushB = [True]

                def open_chunkB():
                    stagB["Y"] = spoolB.tile([128, NS, 128], F32, tag="ystag", name="ystag")

                def flush_chunkB():
                    ch = cur_chunkB[0]
                    nb = schedB.chunk_sizes[ch]
                    yst = stagB["Y"]
                    mul = nc.vector.tensor_tensor(
                        out=_ap(yst[:], 0, [[128, nb], [32, H], [1, C]]),
                        in0=_ap(yst[:], 0, [[128, nb], [32, H], [1, C]]),
                        in1=_ap(explg[:], ch * NS * 64, [[64, nb], [1, H], [0, C]]),
                        op=ALU.mult)
                    if first_flushB[0]:
                        s, v = exsem.cur(0)
                        mul.wait_op(s, v, "sem-ge")
                        first_flushB[0] = False
                    sem = ysem.bump(0)
                    nc.gpsimd.dma_scatter_add(
                        y[:], yst[:, :nb, :],
                        scB_sb[:, sc_offB[0]:sc_offB[0] + nb * 8],
                        nb * 128, nb * 128, 128,
                        prepare_only=True, sem=sem,
                        single_packet=False, queue_num=0)
                    nc.gpsimd.trigger_dma(count=None, queue_num=0)
                    sc_offB[0] += nb * 8
                    cur_chunkB[0] += 1

                open_chunkB()
                tile_last_chunk = {}
                for g in schedB.groups:
                    tile_last_chunk[g[0]] = max(tile_last_chunk.get(g[0], 0), g[5])
                for t in range(schedB.ntiles):
                    G = gpoolB.tile([128, TC, 128], BF16, tag="G")
                    slot = gsemB.acquire()
                    q = gsemB.queues[slot]
                    ch_last = tile_last_chunk.get(t, 0)
                    for si, (c0s, cc) in enumerate(_subcols(TC)):
                        sem = gsemB.bump(slot)
                        nc.gpsimd.dma_gather(
                            G[:, c0s:c0s + cc, :], EAp[:],
                            gB_sb[:, t * TC * 8 + c0s * 8:
                                  t * TC * 8 + (c0s + cc) * 8],
                            cc * 128, cc * 128, 128,
                            prepare_only=True, sem=sem,
                            single_packet=False, queue_num=q)
                        trig = nc.gpsimd.trigger_dma(count=None, queue_num=q)
                        if si == 0 and ch_last >= 4:
                            # WAR for yst (bufs=4): gate the tile's gather
                            # DMAs on chunk ch_last-4's y-scatter having
                            # drained. Trigger-side wait is deadlock-safe:
                            # the awaited DMA's doorbell was already rung
                            # by an earlier in-order Pool trigger.
                            trig.wait_op(ysem.sems[0], 16 * (ch_last - 3), "sem-ge")
                    for (_, c0, k, B, b0, ch) in groups_by_tileB.get(t, []):
                        if ch != cur_chunkB[0]:
                            flush_chunkB()
                            open_chunkB()
                        bpos = b0 - ch * NS
                        red = nc.vector.tensor_reduce(
                            out=stagB["Y"][:, bpos:bpos + B, :],
                            in_=_ap(G[:], c0 * 128, [[k * 128, B], [1, 128], [128, k]]),
                            axis=AX.X, op=ALU.add)
                        s, v = gsemB.cur(slot)
                        red.wait_op(s, v, "sem-ge")
                flush_chunkB()
                # terminal guard: kernel end must not race the last y
                # scatter DMAs; hang the wait on a real consumer of yst.
                guard = spoolB.tile([128, 1, 128], F32, tag="guard", name="guard")
                g = nc.vector.tensor_copy(guard[:, 0, :], stagB["Y"][:, 0, :])
                s, v = ysem.cur(0)
                g.wait_op(s, v, "sem-ge")
                for p in reversed(pb_stack):
                    p.__exit__(None, None, None)
    nc.compile()
    return nc


def host_inputs(cfg: Cfg, x, ha, W_node, W_edge, attn_l, bias, streams):
    x = np.asarray(x, np.float32)
    ha = np.asarray(ha, np.float32)
    W_node = np.asarray(W_node, np.float32)
    W_edge = np.asarray(W_edge, np.float32)
    attn_flat = np.asarray(attn_l, np.float32).reshape(-1)
    bias = np.asarray(bias, np.float32).reshape(-1)
    attn_rep = np.tile(attn_flat[None, :], (128, 1))
    bias_t = np.tile(bias[None, :], (128, 16))
    ha_pad = np.zeros((cfg.ET_ROWS, cfg.D), np.float32)
    ha_pad[:cfg.E] = ha
    haT = np.ascontiguousarray(ha_pad.T)
    in_maps = []
    for m in range(cfg.NC):
        xs = np.zeros((cfg.NT_ROWS, cfg.D), np.float32)
        xs[:cfg.NSH] = x[m * cfg.NSH:(m + 1) * cfg.NSH]
        st = streams[m]
        in_maps.append({
            "xT": np.ascontiguousarray(xs.T),
            "haT": haT,
            "Wn": W_node, "We": W_edge,
            "attn": attn_rep, "bias_t": bias_t,
            "gA_i": st["gA"], "scA_i": st["scA"],
            "gB_i": st["gB"], "scB_i": st["scB"], "rB_i": st["rB"],
        })
    return in_maps


# ======================== public entry point ========================
_CFG = Cfg()
LAST_RESULTS = None


def _install_axon_ntff_shim():
    import sys, types, ctypes, contextlib
    import concourse.bass_utils as bu
    bu.upload_artifacts = lambda d: str(d)
    try:
        from antenv.axon_hooks import get_axon_ntff_profile_hook  # noqa
        return
    except ImportError:
        pass
    so_path = "/opt/axon/libaxon_pjrt.so"
    try:
        lib = ctypes.CDLL(so_path)
    except OSError:
        return
    if not hasattr(lib, "axon_start_nrt_profile"):
        return
    lib.axon_start_nrt_profile.argtypes = [ctypes.POINTER(ctypes.c_int64),
                                           ctypes.c_size_t]
    lib.axon_start_nrt_profile.restype = ctypes.c_int64
    lib.axon_stop_nrt_profile.argtypes = [ctypes.c_char_p]
    lib.axon_stop_nrt_profile.restype = ctypes.c_int64

    @contextlib.contextmanager
    def _hook(output_dir, device_ids):
        import jax
        jax.devices()
        if device_ids:
            ids = (ctypes.c_int64 * len(device_ids))(*device_ids)
            rc = lib.axon_start_nrt_profile(ids, len(device_ids))
        else:
            rc = lib.axon_start_nrt_profile(None, 0)
        if rc != 0:
            raise RuntimeError(f"axon_start_nrt_profile rc={rc}")
        try:
            yield
        finally:
            n = lib.axon_stop_nrt_profile(str(output_dir).encode())
            print(f"ntff profile: {n} file(s) -> {output_dir}")

    mod = types.ModuleType("antenv.axon_hooks")
    mod.get_axon_ntff_profile_hook = lambda: _hook
    mod.set_axon_ntff_profile_hook = lambda h: None
    sys.modules["antenv.axon_hooks"] = mod


def kernel(**inputs) -> np.ndarray:
    import os
    from concourse.bass_utils import run_bass_kernel_spmd
    cfg = _CFG
    x = np.asarray(inputs["x"], np.float32)
    ha = np.asarray(inputs["hyperedge_attr"], np.float32)
    node_idx = np.asarray(inputs["node_idx"]).astype(np.int64)
    edge_idx = np.asarray(inputs["edge_idx"]).astype(np.int64)
    schedA, schedB, streams = build_plan(node_idx, edge_idx, cfg)
    nc = build_bass(cfg, schedA, schedB, [list(range(cfg.NC))])
    in_maps = host_inputs(cfg, x, ha, inputs["W_node"], inputs["W_edge"],
                          inputs["attn_l"], inputs["bias"], streams)
    trace = os.environ.get("GNN_TRACE", "0") == "1"
    if trace:
        _install_axon_ntff_shim()
    res = run_bass_kernel_spmd(nc, in_maps, list(range(cfg.NC)), trace=trace)
    global LAST_RESULTS
    LAST_RESULTS = res
    out = np.concatenate(
        [np.asarray(res.results[m]["y"])[:cfg.NSH] for m in range(cfg.NC)], axis=0)
    return np.ascontiguousarray(out, dtype=np.float32)


# revision 32
# speedup vs baseline: 1.6511x; 1.4788x over previous
"""Hypergraph conv kernel, v2.

Pipeline (node-sharded, 8 cores):
  phase1: nfw_table[n] = bf16([exp(a_n)*nf_n (128) | exp(a_n) (4) | pad]),
          expl_table[n] = f32 exp(a_n) (for pass B).
  passA:  per edge-run batch: gather nfw rows, ONE reduce -> [U|D] partial,
          cast bf16, scatter-add into U_table[ET,256] (cols 0:132).
  AR:     one bf16 AllReduce of U_table.
  EA:     EAp[e] = bf16((U/D + ef)/D)   (ef projected during passA window)
  passB:  gather EAp rows per incidence, ONE reduce per run group,
          multiply by expl per chunk, scatter-add into y.

SWDGE gathers/scatters run gen_mode=0: descriptor emission on the Pool
Q7 (~3.3ns/row) is the serial spine; transfers overlap it (engine
releases at doorbell; Tile syncs consumers on the DMA sem).
"""
import numpy as np
from dataclasses import dataclass

import concourse.bass as bass
import concourse.mybir as mybir
import concourse.bacc as bacc
import concourse.tile as tile
import bass_rust
from concourse.library_config import mlp as mlp_library
from concourse._compat import get_trn_type, cdiv

F32 = mybir.dt.float32
BF16 = mybir.dt.bfloat16
I16 = mybir.dt.int16
AX = mybir.AxisListType
ALU = mybir.AluOpType
ACTF = mybir.ActivationFunctionType

ROW = 256          # nfw/U table row width (bf16 elems); cols 0:128 nfw, 128:132 expl
UCOL = 132         # useful cols in nfw/U rows


@dataclass
class Cfg:
    N: int = 100000
    E: int = 25000
    D: int = 128
    H: int = 4
    C: int = 32
    NC: int = 8
    TILE_COLS: int = 30
    NSTAGE: int = 14

    @property
    def NSH(self):
        return self.N // self.NC

    @property
    def NT_ROWS(self):
        return cdiv(self.NSH + 1, 128) * 128

    @property
    def ET_ROWS(self):
        return cdiv(self.E + 1, 128) * 128

    @property
    def DUMMY_NODE(self):
        return self.NSH

    @property
    def JUNK_EDGE(self):
        return self.E


def _runs(keys):
    if len(keys) == 0:
        return (np.zeros(0, np.int64),) * 3
    change = np.flatnonzero(np.diff(keys)) + 1
    starts = np.concatenate([[0], change]).astype(np.int64)
    ends = np.concatenate([change, [len(keys)]]).astype(np.int64)
    return starts, ends - starts, keys[starts].astype(np.int64)


@dataclass
class Sched:
    batches: list          # [(k, tile, c0)]
    groups: list           # [(tile, c0, k, B, b0, chunk)]
    ntiles: int
    nchunks: int
    nbatch: int
    chunk_sizes: list


def _mk_schedule(lens_list, cfg: Cfg) -> Sched:
    sorted_lens = [np.sort(np.asarray(l))[::-1] for l in lens_list]
    nbatch_total = max(cdiv(len(l), 128) for l in sorted_lens)
    batches = []
    for b in range(nbatch_total):
        w = 1
        for ls in sorted_lens:
            if b * 128 < len(ls):
                w = max(w, int(ls[b * 128]))
        batches.append(w)
    assert max(batches) <= cfg.TILE_COLS, \
        f"run length {max(batches)} > TILE_COLS"
    placed = []
    t, c = 0, 0
    for k in batches:
        if c + k > cfg.TILE_COLS:
            t += 1
            c = 0
        placed.append((k, t, c))
        c += k
    ntiles = t + 1 if placed else 1
    nbatch = len(placed)
    nchunks = cdiv(nbatch, cfg.NSTAGE)
    chunk_sizes = [min(cfg.NSTAGE, nbatch - i * cfg.NSTAGE) for i in range(nchunks)]
    groups = []
    for bi, (k, t, c0) in enumerate(placed):
        ch = bi // cfg.NSTAGE
        if groups and groups[-1][0] == t and groups[-1][2] == k \
                and groups[-1][5] == ch \
                and groups[-1][1] + groups[-1][2] * groups[-1][3] == c0 \
                and groups[-1][4] + groups[-1][3] == bi:
            t0, c0g, kg, B, b0, chg = groups[-1]
            groups[-1] = (t0, c0g, kg, B + 1, b0, chg)
        else:
            groups.append((t, c0, k, 1, bi, ch))
    return Sched(placed, groups, ntiles, nchunks, nbatch, chunk_sizes)


def _wrap16(flat):
    assert len(flat) % 16 == 0
    b = flat.reshape(-1, 16).T.astype(np.int16)
    return np.tile(b, (8, 1))


SUBMAX = 30


def _subcols(n):
    return [(i, min(SUBMAX, n - i)) for i in range(0, n, SUBMAX)]


def _mk_streams(sched: Sched, starts, lens, gvals, svals, runvals,
                dummy_g, junk_s, dummy_run, cfg: Cfg):
    TC = cfg.TILE_COLS
    g_arr = np.full((sched.ntiles, TC, 128), dummy_g, np.int64)
    s_arr = np.full((sched.nbatch, 128), junk_s, np.int64)
    r_arr = np.full((sched.nbatch, 128), dummy_run, np.int64)
    order = np.argsort(-lens, kind="stable") if len(lens) else np.zeros(0, np.int64)
    for bi, (k, t, c0) in enumerate(sched.batches):
        idxs = order[bi * 128:(bi + 1) * 128]
        nr = len(idxs)
        if nr:
            st = starts[idxs]
            kr = lens[idxs]
            assert kr[0] <= k
            for kk in np.unique(kr):
                sel = np.flatnonzero(kr == kk)
                gm = gvals[st[sel][None, :] + np.arange(kk)[:, None]]
                g_arr[t, c0:c0 + kk, sel] = gm.T
            s_arr[bi, :nr] = svals[idxs]
            r_arr[bi, :nr] = runvals[idxs]
    g_idx = np.concatenate(
        [_wrap16(g_arr[t, c0:c0 + cc].reshape(-1))
         for t in range(sched.ntiles) for (c0, cc) in _subcols(TC)], axis=1)
    sc_blocks = []
    off = 0
    for nb in sched.chunk_sizes:
        sc_blocks.append(_wrap16(s_arr[off:off + nb].reshape(-1)))
        off += nb
    sc_idx = np.concatenate(sc_blocks, axis=1)
    r_idx = np.concatenate(
        [_wrap16(r_arr[b0:b0 + bb].reshape(-1))
         for (b0, bb) in _subcols(sched.nbatch)], axis=1)
    return g_idx, sc_idx, r_idx


def build_plan(node_idx, edge_idx, cfg: Cfg):
    node_idx = np.asarray(node_idx).astype(np.int64)
    edge_idx = np.asarray(edge_idx).astype(np.int64)
    percore = []
    for m in range(cfg.NC):
        sel = np.flatnonzero(node_idx // cfg.NSH == m)
        nl = node_idx[sel] - m * cfg.NSH
        eg = edge_idx[sel]
        sA, lA, vA = _runs(eg)
        oB = np.argsort(nl, kind="stable")
        nB = nl[oB]
        eB = eg[oB]
        sB, lB, vB = _runs(nB)
        percore.append(dict(nl=nl, eg=eg, sA=sA, lA=lA, vA=vA,
                            eB=eB, sB=sB, lB=lB, vB=vB))
    schedA = _mk_schedule([c["lA"] for c in percore], cfg)
    schedB = _mk_schedule([c["lB"] for c in percore], cfg)
    streams = []
    for c in percore:
        gA, scA, _ = _mk_streams(
            schedA, c["sA"], c["lA"],
            gvals=c["nl"], svals=c["vA"], runvals=c["vA"],
            dummy_g=cfg.DUMMY_NODE, junk_s=cfg.JUNK_EDGE,
            dummy_run=cfg.JUNK_EDGE, cfg=cfg)
        gB, scB, rB = _mk_streams(
            schedB, c["sB"], c["lB"],
            gvals=c["eB"], svals=c["vB"], runvals=c["vB"],
            dummy_g=cfg.JUNK_EDGE, junk_s=cfg.DUMMY_NODE,
            dummy_run=cfg.DUMMY_NODE, cfg=cfg)
        streams.append(dict(gA=gA, scA=scA, gB=gB, scB=scB, rB=rB))
    return schedA, schedB, streams


def _ap(t_ap, off, dims):
    base = t_ap
    part = base.ap[0]
    return bass_rust.AP(base.tensor, base.offset + off, [part] + dims)


def build_bass(cfg: Cfg, schedA: Sched, schedB: Sched, replica_groups):
    import os
    _stops = ["init", "phase1", "passA", "coll", "ea", "full"]
    _stop = _stops.index(os.environ.get("GNN_STOP", "full"))
    TC, NS = cfg.TILE_COLS, cfg.NSTAGE
    H, C = cfg.H, cfg.C
    D = cfg.D
    NT, ET = cfg.NT_ROWS, cfg.ET_ROWS
    NSH, E = cfg.NSH, cfg.E
    n_a_node = NT // 128
    n_a_edge = ET // 128

    nc = bacc.Bacc(get_trn_type() or "TRN2", target_bir_lowering=False, debug=False,
                   num_swdge_queues=4)
    _qrr = [0]

    def _q():
        q = _qrr[0] % 4
        _qrr[0] += 1
        return q

    # ---- I/O ----
    xT = nc.dram_tensor("xT", [D, NT], F32, kind="ExternalInput")
    haT = nc.dram_tensor("haT", [D, ET], F32, kind="ExternalInput")
    Wn = nc.dram_tensor("Wn", [D, H * C], F32, kind="ExternalInput")
    We = nc.dram_tensor("We", [D, H * C], F32, kind="ExternalInput")
    attn = nc.dram_tensor("attn", [128, H * C], F32, kind="ExternalInput")
    bias_t = nc.dram_tensor("bias_t", [128, 16 * H * C], F32, kind="ExternalInput")
    gA_i = nc.dram_tensor("gA_i", [128, schedA.ntiles * TC * 8], I16, kind="ExternalInput")
    scA_i = nc.dram_tensor("scA_i", [128, schedA.nbatch * 8], I16, kind="ExternalInput")
    gB_i = nc.dram_tensor("gB_i", [128, schedB.ntiles * TC * 8], I16, kind="ExternalInput")
    scB_i = nc.dram_tensor("scB_i", [128, schedB.nbatch * 8], I16, kind="ExternalInput")
    rB_i = nc.dram_tensor("rB_i", [128, schedB.nbatch * 8], I16, kind="ExternalInput")
    y = nc.dram_tensor("y", [NT, H * C], F32, kind="ExternalOutput")
    _dbg = os.environ.get("GNN_DEBUG_OUTS", "0") == "1"
    if _dbg:
        nfw_dbg = nc.dram_tensor("nfw_dbg", [NT, ROW], BF16, kind="ExternalOutput")
        U_dbg = nc.dram_tensor("U_dbg", [ET, ROW], BF16, kind="ExternalOutput")
        Ured_dbg = nc.dram_tensor("Ured_dbg", [ET, ROW], BF16, kind="ExternalOutput")
        EAp_dbg = nc.dram_tensor("EAp_dbg", [ET, 128], BF16, kind="ExternalOutput")

    # ---- internal DRAM ----
    nfw_table = nc.dram_tensor("nfw_table", [NT, ROW], BF16)
    expl_table = nc.dram_tensor("expl_table", [NT, 64], F32)
    ef_table = nc.dram_tensor("ef_table", [ET, 128], BF16)
    U_table = nc.dram_tensor("U_table", [ET, ROW], BF16)
    U_red = nc.dram_tensor("U_red", [ET, ROW], BF16, addr_space="Shared")
    EAp = nc.dram_tensor("EAp", [ET, 128], BF16)

    nfw_v = nfw_table[:].rearrange("(a p) c -> p a c", p=128)
    expl_v = expl_table[:].rearrange("(a p) c -> p a c", p=128)
    ef_v = ef_table[:].rearrange("(a p) c -> p a c", p=128)
    U_v = U_table[:].rearrange("(a p) c -> p a c", p=128)
    Ured_v = U_red[:].rearrange("(a p) c -> p a c", p=128)
    EAp_v = EAp[:].rearrange("(a p) c -> p a c", p=128)
    y_v = y[:].rearrange("(a p) c -> p a c", p=128)

    # SWDGE gathers/scatters run gen_mode=0: the Pool engine emits
    # descriptors (~3.3ns/row) and releases at doorbell; the DMA-completion
    # sem is attached by Tile, so consumer/WAR sync is fully tracked.
    with tile.TileContext(nc) as tc:
        with tc.tile_pool(name="const", bufs=1) as cpool:
            nc.gpsimd.load_library(mlp_library)
            Wn_sb = cpool.tile([D, H * C], F32)
            We_sb = cpool.tile([D, H * C], F32)
            attn_sb = cpool.tile([128, H * C], F32)
            bias_sb = cpool.tile([128, 16 * H * C], F32)
            zbf = cpool.tile([128, 16 * ROW], BF16)
            nc.sync.dma_start(Wn_sb[:], Wn[:])
            nc.sync.dma_start(We_sb[:], We[:])
            nc.sync.dma_start(attn_sb[:], attn[:])
            nc.sync.dma_start(bias_sb[:], bias_t[:])
            nc.gpsimd.memset(zbf[:], 0.0)

            # ---------- init: U=0 (scalar HWDGE queue), y=bias ----------
            zv = zbf[:].rearrange("p (a c) -> p a c", c=ROW)
            for a0 in range(0, n_a_edge, 16):
                aa = min(16, n_a_edge - a0)
                nc.scalar.dma_start(U_v[:, a0:a0 + aa, :], zv[:, :aa, :])
            bv = bias_sb[:].rearrange("p (a c) -> p a c", c=H * C)
            for a0 in range(0, n_a_node, 16):
                aa = min(16, n_a_node - a0)
                nc.scalar.dma_start(y_v[:, a0:a0 + aa, :], bv[:, :aa, :])

            # ---------- phase 1: node projection -> nfw + expl ----------
            if _stop >= 1:
                with (tc.tile_pool(name="p1", bufs=2) as p1,
                    tc.tile_pool(name="p1s", bufs=2) as p1s,
                    tc.tile_pool(name="ps", bufs=4, space="PSUM") as psp):
                  CHK = 16
                  for a0 in range(0, n_a_node, CHK):
                      aa = min(CHK, n_a_node - a0)
                      xc = p1.tile([D, CHK * 128], F32, tag="xc")
                      nc.sync.dma_start(xc[:, :aa * 128], xT[:, a0 * 128:(a0 + aa) * 128])
                      nfst = p1s.tile([128, CHK, 128], F32, tag="nfst")
                      for i in range(aa):
                          mm = psp.tile([128, 128], F32, tag="mm")
                          nc.tensor.matmul(mm[:], xc[:, i * 128:(i + 1) * 128], Wn_sb[:],
                                           start=True, stop=True)
                          nc.vector.tensor_copy(nfst[:, i, :], mm[:])
                      tmp1 = p1s.tile([128, CHK, 128], F32, tag="tmp1")
                      nc.vector.tensor_tensor(
                          out=tmp1[:, :aa, :].rearrange("p a (h c) -> p a h c", h=H),
                          in0=nfst[:, :aa, :].rearrange("p a (h c) -> p a h c", h=H),
                          in1=attn_sb[:].rearrange("p (h c) -> p h c", h=H)
                              .unsqueeze(1).broadcast_to([128, aa, H, C]),
                          op=ALU.mult)
                      praw = p1s.tile([128, CHK, H], F32, tag="praw")
                      nc.vector.tensor_reduce(
                          out=praw[:, :aa, :],
                          in_=tmp1[:, :aa, :].rearrange("p a (h c) -> p a h c", h=H),
                          axis=AX.X, op=ALU.add)
                      est = p1s.tile([128, CHK, 64], F32, tag="est")
                      nc.gpsimd.memset(est[:], 0.0)
                      nc.scalar.activation(est[:, :aa, 0:H], praw[:, :aa, :], ACTF.Exp)
                      nrow = p1s.tile([128, CHK, ROW], BF16, tag="nrow")
                      nc.gpsimd.memset(nrow[:], 0.0)
                      nc.vector.tensor_tensor(
                          out=nrow[:, :aa, 0:128].rearrange("p a (h c) -> p a h c", h=H),
                          in0=nfst[:, :aa, :].rearrange("p a (h c) -> p a h c", h=H),
                          in1=est[:, :aa, 0:H].unsqueeze(3).broadcast_to([128, aa, H, C]),
                          op=ALU.mult)
                      nc.vector.tensor_copy(nrow[:, :aa, 128:UCOL], est[:, :aa, 0:H])
                      nc.sync.dma_start(nfw_v[:, a0:a0 + aa, :], nrow[:, :aa, :])
                      nc.sync.dma_start(expl_v[:, a0:a0 + aa, :], est[:, :aa, :])
                  # zero dummy/pad rows (incl. expl cols -> no D contribution)
                  nc.sync.dma_start(nfw_table[NSH:NT, :], zbf[0:NT - NSH, 0:ROW])

            # ---------- pass A ----------
            if _stop >= 2:
                # staging bufs=4 makes buffer-reuse WAR sound with NO manual
                # waits: a chunk-ch writer is RAW-gated on reduces, which wait
                # gathers enqueued (queue-0 FIFO) after flush(ch-s-1), s<=3
                # being the max chunks one tile spans; FIFO order then implies
                # scatter(ch-4) has fully drained.
                _span = {}
                for g in schedA.groups:
                    _span.setdefault(g[0], []).append(g[5])
                assert all(max(v) - min(v) <= 2 for v in _span.values()), "tile spans >3 chunks"
                with (tc.tile_pool(name="gA", bufs=4) as gpool,
                    tc.tile_pool(name="stA", bufs=4) as spool,
                    tc.tile_pool(name="idxA", bufs=1) as ipool):
                  gA_sb = ipool.tile([128, schedA.ntiles * TC * 8], I16)
                  scA_sb = ipool.tile([128, schedA.nbatch * 8], I16)
                  nc.sync.dma_start(gA_sb[:], gA_i[:])
                  nc.sync.dma_start(scA_sb[:], scA_i[:])

                  groups_by_tile = {}
                  for g in schedA.groups:
                      groups_by_tile.setdefault(g[0], []).append(g)

                  cur_chunk = [0]
                  stag = {}
                  sc_off = [0]

                  def open_chunk():
                      stag["U"] = spool.tile([128, NS, UCOL], F32, tag="ustag", name="ustag")

                  def flush_chunk():
                      ch = cur_chunk[0]
                      nb = schedA.chunk_sizes[ch]
                      # full 512B rows (scatter payloads must stay on the
                      # proven 256B-multiple geometry); pad cols add zero.
                      # Pads are zeroed once per buffer instance (first 4
                      # chunks) and never dirtied afterwards.
                      ubf = spool.tile([128, NS, ROW], BF16, tag="ubf", name="ubf")
                      if ch < 4:
                          nc.gpsimd.memset(ubf[:, :, UCOL:], 0.0)
                      nc.vector.tensor_copy(ubf[:, :nb, 0:UCOL], stag["U"][:, :nb, :])
                      nc.gpsimd.dma_scatter_add(
                          U_table[:], ubf[:, :nb, :],
                          scA_sb[:, sc_off[0]:sc_off[0] + nb * 8],
                          nb * 128, nb * 128, ROW,
                          single_packet=False, queue_num=_q())
                      sc_off[0] += nb * 8
                      cur_chunk[0] += 1

                  open_chunk()
                  for t in range(schedA.ntiles):
                      G = gpool.tile([128, TC, ROW], BF16, tag="G")
                      for (c0s, cc) in _subcols(TC):
                          nc.gpsimd.dma_gather(
                              G[:, c0s:c0s + cc, :], nfw_table[:],
                              gA_sb[:, t * TC * 8 + c0s * 8:
                                    t * TC * 8 + (c0s + cc) * 8],
                              cc * 128, cc * 128, ROW,
                              single_packet=False, queue_num=_q())
                      for (_, c0, k, B, b0, ch) in groups_by_tile.get(t, []):
                          if ch != cur_chunk[0]:
                              flush_chunk()
                              open_chunk()
                          bpos = b0 - ch * NS
                          nc.vector.tensor_reduce(
                              out=stag["U"][:, bpos:bpos + B, :],
                              in_=_ap(G[:], c0 * ROW, [[k * ROW, B], [1, UCOL], [ROW, k]]),
                              axis=AX.X, op=ALU.add)
                  flush_chunk()

            # ---------- edge projection (overlaps passA tail / AR) ----------
            if _stop >= 1:
                with (tc.tile_pool(name="pe", bufs=2) as pe,
                    tc.tile_pool(name="pes", bufs=2) as pes,
                    tc.tile_pool(name="ps2", bufs=4, space="PSUM") as psp2):
                  CHK = 16
                  for a0 in range(0, n_a_edge, CHK):
                      aa = min(CHK, n_a_edge - a0)
                      hc = pe.tile([D, CHK * 128], F32, tag="hc")
                      nc.sync.dma_start(hc[:, :aa * 128], haT[:, a0 * 128:(a0 + aa) * 128])
                      efst = pes.tile([128, CHK, 128], BF16, tag="efst")
                      for i in range(aa):
                          mm = psp2.tile([128, 128], F32, tag="mm")
                          nc.tensor.matmul(mm[:], hc[:, i * 128:(i + 1) * 128], We_sb[:],
                                           start=True, stop=True)
                          nc.vector.tensor_copy(efst[:, i, :], mm[:])
                      nc.sync.dma_start(ef_v[:, a0:a0 + aa, :], efst[:, :aa, :])

            # ---------- pass B prologue: index/expl loads (overlap AR) ----------
            if _stop >= 5:
                pb_stack = [tc.tile_pool(name="gB", bufs=6),
                            tc.tile_pool(name="stB", bufs=4),
                            tc.tile_pool(name="idxB", bufs=1)]
                gpoolB, spoolB, ipoolB = [p.__enter__() for p in pb_stack]
                gB_sb = ipoolB.tile([128, schedB.ntiles * TC * 8], I16)
                scB_sb = ipoolB.tile([128, schedB.nbatch * 8], I16)
                rB_sb = ipoolB.tile([128, schedB.nbatch * 8], I16)
                nc.sync.dma_start(gB_sb[:], gB_i[:])
                nc.sync.dma_start(scB_sb[:], scB_i[:])
                nc.sync.dma_start(rB_sb[:], rB_i[:])
                explg = ipoolB.tile([128, schedB.nbatch, 64], F32)
                for (b0s, bb) in _subcols(schedB.nbatch):
                    nc.gpsimd.dma_gather(
                        explg[:, b0s:b0s + bb, :], expl_table[:],
                        rB_sb[:, b0s * 8:(b0s + bb) * 8],
                        bb * 128, bb * 128, 64,
                        single_packet=False, queue_num=_q())

            # ---------- collective ----------
            if _stop >= 3:
                nc.gpsimd.collective_compute(
                    "AllReduce", ALU.add, replica_groups=replica_groups,
                    ins=[U_table[:]], outs=[U_red[:]])

            # ---------- EA' = (U/D + ef)/D ----------
            if _stop >= 4:
                with tc.tile_pool(name="ea", bufs=2) as eap:
                  for a0 in range(0, n_a_edge, 16):
                      aa = min(16, n_a_edge - a0)
                      uc = eap.tile([128, 16, ROW], BF16, tag="uc")
                      efc = eap.tile([128, 16, 128], BF16, tag="efc")
                      nc.sync.dma_start(uc[:, :aa, :], Ured_v[:, a0:a0 + aa, :])
                      nc.sync.dma_start(efc[:, :aa, :], ef_v[:, a0:a0 + aa, :])
                      dv = eap.tile([128, 16, H], F32, tag="dv")
                      nc.vector.tensor_copy(dv[:, :aa, :], uc[:, :aa, 128:UCOL])
                      nc.vector.tensor_scalar_add(dv[:, :aa, :], dv[:, :aa, :], 1e-30)
                      inv = eap.tile([128, 16, H], F32, tag="inv")
                      nc.vector.reciprocal(inv[:, :aa, :], dv[:, :aa, :])
                      inv_b = inv[:, :aa, :].unsqueeze(3).broadcast_to([128, aa, H, C])
                      t1 = eap.tile([128, 16, 128], F32, tag="t1")
                      nc.vector.tensor_tensor(
                          out=t1[:, :aa, :].rearrange("p a (h c) -> p a h c", h=H),
                          in0=uc[:, :aa, 0:128].rearrange("p a (h c) -> p a h c", h=H),
                          in1=inv_b, op=ALU.mult)
                      nc.vector.tensor_tensor(out=t1[:, :aa, :], in0=t1[:, :aa, :],
                                              in1=efc[:, :aa, :], op=ALU.add)
                      eab = eap.tile([128, 16, 128], BF16, tag="eab")
                      nc.vector.tensor_tensor(
                          out=eab[:, :aa, :].rearrange("p a (h c) -> p a h c", h=H),
                          in0=t1[:, :aa, :].rearrange("p a (h c) -> p a h c", h=H),
                          in1=inv_b, op=ALU.mult)
                      nc.sync.dma_start(EAp_v[:, a0:a0 + aa, :], eab[:, :aa, :])
                  nj = ET - E
                  nc.sync.dma_start(EAp[E:ET, :], zbf[0:nj, 0:128])

            if _dbg:
                if _stop >= 1:
                    nc.sync.dma_start(nfw_dbg[:], nfw_table[:])
                if _stop >= 2:
                    nc.sync.dma_start(U_dbg[:], U_table[:])
                if _stop >= 3:
                    nc.sync.dma_start(Ured_dbg[:], U_red[:])
                if _stop >= 4:
                    nc.sync.dma_start(EAp_dbg[:], EAp[:])

            # ---------- pass B ----------
            if _stop >= 5:
                groups_by_tileB = {}
                for g in schedB.groups:
                    groups_by_tileB.setdefault(g[0], []).append(g)
                cur_chunkB = [0]
                stagB = {}
                sc_offB = [0]

                def open_chunkB():
                    stagB["Y"] = spoolB.tile([128, NS, 128], F32, tag="ystag", name="ystag")

                def flush_chunkB():
                    ch = cur_chunkB[0]
                    nb = schedB.chunk_sizes[ch]
                    yst = stagB["Y"]
                    nc.vector.tensor_tensor(
                        out=_ap(yst[:], 0, [[128, nb], [32, H], [1, C]]),
                        in0=_ap(yst[:], 0, [[128, nb], [32, H], [1, C]]),
                        in1=_ap(explg[:], ch * NS * 64, [[64, nb], [1, H], [0, C]]),
                        op=ALU.mult)
                    nc.gpsimd.dma_scatter_add(
                        y[:], yst[:, :nb, :],
                        scB_sb[:, sc_offB[0]:sc_offB[0] + nb * 8],
                        nb * 128, nb * 128, 128,
                        single_packet=False, queue_num=_q())
                    sc_offB[0] += nb * 8
                    cur_chunkB[0] += 1

                open_chunkB()
                for t in range(schedB.ntiles):
                    G = gpoolB.tile([128, TC, 128], BF16, tag="G")
                    for (c0s, cc) in _subcols(TC):
                        nc.gpsimd.dma_gather(
                            G[:, c0s:c0s + cc, :], EAp[:],
                            gB_sb[:, t * TC * 8 + c0s * 8:
                                  t * TC * 8 + (c0s + cc) * 8],
                            cc * 128, cc * 128, 128,
                            single_packet=False, queue_num=_q())
                    for (_, c0, k, B, b0, ch) in groups_by_tileB.get(t, []):
                        if ch != cur_chunkB[0]:
                            flush_chunkB()
                            open_chunkB()
                        bpos = b0 - ch * NS
                        nc.vector.tensor_reduce(
                            out=stagB["Y"][:, bpos:bpos + B, :],
                            in_=_ap(G[:], c0 * 128, [[k * 128, B], [1, 128], [128, k]]),
                            axis=AX.X, op=ALU.add)
                flush_chunkB()
                for p in reversed(pb_stack):
                    p.__exit__(None, None, None)
    nc.compile()
    return nc


def host_inputs(cfg: Cfg, x, ha, W_node, W_edge, attn_l, bias, streams):
    x = np.asarray(x, np.float32)
    ha = np.asarray(ha, np.float32)
    W_node = np.asarray(W_node, np.float32)
    W_edge = np.asarray(W_edge, np.float32)
    attn_flat = np.asarray(attn_l, np.float32).reshape(-1)
    bias = np.asarray(bias, np.float32).reshape(-1)
    attn_rep = np.tile(attn_flat[None, :], (128, 1))
    bias_t = np.tile(bias[None, :], (128, 16))
    ha_pad = np.zeros((cfg.ET_ROWS, cfg.D), np.float32)
    ha_pad[:cfg.E] = ha
    haT = np.ascontiguousarray(ha_pad.T)
    in_maps = []
    for m in range(cfg.NC):
        xs = np.zeros((cfg.NT_ROWS, cfg.D), np.float32)
        xs[:cfg.NSH] = x[m * cfg.NSH:(m + 1) * cfg.NSH]
        st = streams[m]
        in_maps.append({
            "xT": np.ascontiguousarray(xs.T),
            "haT": haT,
            "Wn": W_node, "We": W_edge,
            "attn": attn_rep, "bias_t": bias_t,
            "gA_i": st["gA"], "scA_i": st["scA"],
            "gB_i": st["gB"], "scB_i": st["scB"], "rB_i": st["rB"],
        })
    return in_maps


# ======================== public entry point ========================
_CFG = Cfg()
LAST_RESULTS = None


def _install_axon_ntff_shim():
    import sys, types, ctypes, contextlib
    import concourse.bass_utils as bu
    bu.upload_artifacts = lambda d: str(d)
    try:
        from antenv.axon_hooks import get_axon_ntff_profile_hook  # noqa
        return
    except ImportError:
        pass
    so_path = "/opt/axon/libaxon_pjrt.so"
    try:
        lib = ctypes.CDLL(so_path)
    except OSError:
        return
    if not hasattr(lib, "axon_start_nrt_profile"):
        return
    lib.axon_start_nrt_profile.argtypes = [ctypes.POINTER(ctypes.c_int64),
                                           ctypes.c_size_t]
    lib.axon_start_nrt_profile.restype = ctypes.c_int64
    lib.axon_stop_nrt_profile.argtypes = [ctypes.c_char_p]
    lib.axon_stop_nrt_profile.restype = ctypes.c_int64

    @contextlib.contextmanager
    def _hook(output_dir, device_ids):
        import jax
        jax.devices()
        if device_ids:
            ids = (ctypes.c_int64 * len(device_ids))(*device_ids)
            rc = lib.axon_start_nrt_profile(ids, len(device_ids))
        else:
            rc = lib.axon_start_nrt_profile(None, 0)
        if rc != 0:
            raise RuntimeError(f"axon_start_nrt_profile rc={rc}")
        try:
            yield
        finally:
            n = lib.axon_stop_nrt_profile(str(output_dir).encode())
            print(f"ntff profile: {n} file(s) -> {output_dir}")

    mod = types.ModuleType("antenv.axon_hooks")
    mod.get_axon_ntff_profile_hook = lambda: _hook
    mod.set_axon_ntff_profile_hook = lambda h: None
    sys.modules["antenv.axon_hooks"] = mod


def kernel(**inputs) -> np.ndarray:
    import os
    from concourse.bass_utils import run_bass_kernel_spmd
    cfg = _CFG
    x = np.asarray(inputs["x"], np.float32)
    ha = np.asarray(inputs["hyperedge_attr"], np.float32)
    node_idx = np.asarray(inputs["node_idx"]).astype(np.int64)
    edge_idx = np.asarray(inputs["edge_idx"]).astype(np.int64)
    schedA, schedB, streams = build_plan(node_idx, edge_idx, cfg)
    nc = build_bass(cfg, schedA, schedB, [list(range(cfg.NC))])
    in_maps = host_inputs(cfg, x, ha, inputs["W_node"], inputs["W_edge"],
                          inputs["attn_l"], inputs["bias"], streams)
    trace = os.environ.get("GNN_TRACE", "0") == "1"
    if trace:
        _install_axon_ntff_shim()
    res = run_bass_kernel_spmd(nc, in_maps, list(range(cfg.NC)), trace=trace)
    global LAST_RESULTS
    LAST_RESULTS = res
    out = np.concatenate(
        [np.asarray(res.results[m]["y"])[:cfg.NSH] for m in range(cfg.NC)], axis=0)
    return np.ascontiguousarray(out, dtype=np.float32)
